# revision 31
# baseline (speedup 1.0000x reference)
"""Trainium2 Bass kernel for single-head causal attention with RoPE.

Problem: B=4, S=4096, D=2048, H=1.
  out = softmax(causal(rope(q@Wq) @ rope(q@Wk)^T / sqrt(D))) @ (q@Wv) @ Wo

Sharding: 8 cores = 4 batches x 2 groups. Each core owns 4 of the batch's 8
512-row blocks ({7,4,3,0} even cores / {6,5,2,1} odd) — a causal-balanced
split: both cores see 18 real key-steps, padded to a uniform per-slot
schedule TSTEPS=[8,6,4,2] so all cores run one NEFF. K/V projections are
computed only for OWN blocks; the pair exchanges K/V per block-pair through
four 2-rank AllGathers that overlap later projection phases.

All matmuls bf16 (PSUM accumulates fp32). RoPE is reduced to half-rotation by
permuting Wq/Wk columns on the host; the score scale 1/sqrt(D) is folded into
Wq. Attention is transpose-free: scores are computed as S^T[k,q] per block,
softmax runs without max-subtraction, the denominator comes from per-qs
pt^T@ones matmuls accumulated directly in [P,4] orientation, and P^T feeds
the PV matmul directly.

Perf structure (vs the first working version):
- Two rotating weight slots (WA left / WB right): Wv prefetches during
  K-proj, Wq during V-proj, so phase transitions never stall on weight DMA.
- DMA queues split by role: sync=streaming loads, scalar=weight prefetch +
  first-attention-step prefetch, gpsimd=all stores + collectives. This avoids
  head-of-line blocking of latency-critical loads behind data-dependent
  stores.
- Slot 3's Q stays in SBUF (no DRAM round trip); first attention step's K/V
  tiles prefetch during Q-proj.
- The FINAL key-step of every slot is triangle-trimmed: per kc chunk only
  queries >= kc*128 are computed in QK/exp/l/PV (on even cores that step is
  the fully-masked pad step; trimming is a superset of its needs).
- ot accumulates in bf16 (no separate cast tile before the O projection).
"""

import json
import math

import ml_dtypes
import numpy as np

import concourse.bass as bass
import concourse.mybir as mybir
import concourse.tile as tile
from concourse.bass_utils import run_bass_kernel_spmd


def _split_multi_waits(bir_json_bytes):
    """Rewrite BIR so no instruction carries more than one semaphore wait.

    The walrus build in this environment rejects instructions with >1 sync
    wait. Extra waits are hoisted onto injected same-engine EventSemaphore
    instructions placed immediately before the instruction (engine program
    order makes them gate it)."""
    d = json.loads(bir_json_bytes)
    for fn in d["functions"]:
        for blk in fn["blocks"]:
            out = []
            for inst in blk["instructions"]:
                si = inst.get("sync_info") or {}
                ow = si.get("on_wait") or []
                if len(ow) > 1:
                    for i, w in enumerate(ow[:-1]):
                        out.append({
                            "debug": inst.get("debug"),
                            "engine": inst["engine"],
                            "ins": [],
                            "outs": [],
                            "name": f"{inst['name']}_sw{i}",
                            "opcode": "EventSemaphore",
                            "sync_info": {"on_update": [], "on_wait": [w]},
                        })
                    si["on_wait"] = [ow[-1]]
                out.append(inst)
            blk["instructions"] = out
    return json.dumps(d).encode()


def _install_split_waits():
    import concourse.bass_utils as bu
    if getattr(bu, "_split_waits_installed", False):
        return
    orig = bu.compile_bir_kernel

    def patched(bir_json, tmpdir, neff_name="file.neff"):
        return orig(_split_multi_waits(bir_json), tmpdir, neff_name)

    bu.compile_bir_kernel = patched
    bu._split_waits_installed = True
    import concourse.bass2jax as b2j
    if getattr(b2j, "compile_bir_kernel", None) is orig:
        b2j.compile_bir_kernel = patched


_install_split_waits()

BF = mybir.dt.bfloat16
F32 = mybir.dt.float32
bf16 = ml_dtypes.bfloat16

B, S, D = 4, 4096, 2048
HALF = D // 2
P = 128
QB = 512           # query block (one slot)
KB = 512           # key step
NSLOT = 4          # query blocks per core
NQ = NSLOT * QB    # 2048 own queries per core
DI = D // P        # 16 contraction chunks
NCH = D // P       # 16 output chunks
TSTEPS = [8, 6, 4, 2]   # padded key steps per slot (uniform across cores)
SLOT_ORDER = [3, 2, 1, 0]   # processing order: ascending step counts
BLOCKS_EVEN = [7, 4, 3, 0]
BLOCKS_ODD = [6, 5, 2, 1]
KEYS_EVEN = [0, 3, 4, 7]   # same blocks, ascending (AllGather pairing order)
KEYS_ODD = [1, 2, 5, 6]
# key block s of the batch lives at gathered[AGIDX[s][0]][AGIDX[s][1]]
AGIDX = [(0, 0), (0, 1), (1, 1), (1, 0), (2, 0), (2, 1), (3, 1), (3, 0)]
KTSZ = D * KB              # elements of one K^T block [D, 512]
ROPE_BASE = 10000.0
NEG = -1.0e30


def _dma_in(eng, dst, src_ap, n):
    """Per-chunk DMA load of a [P, n, F] tile from a "(c p) f" DRAM view."""
    v = src_ap.rearrange("(c p) f -> p c f", p=P)
    for c in range(n):
        eng.dma_start(dst[:, c], v[:, c])


def _build():
    nc = bass.Bass(num_devices=8)

    qT_own = nc.declare_dram_parameter("qT_own", [D, NQ], BF, isOutput=False)
    qT_keys = nc.declare_dram_parameter("qT_keys", [D, NQ], BF, isOutput=False)
    Wq = nc.declare_dram_parameter("Wq", [D, D], BF, isOutput=False)
    Wk = nc.declare_dram_parameter("Wk", [D, D], BF, isOutput=False)
    Wv = nc.declare_dram_parameter("Wv", [D, D], BF, isOutput=False)
    Wo = nc.declare_dram_parameter("Wo", [D, D], BF, isOutput=False)
    cosO = nc.declare_dram_parameter("cosO", [HALF, NQ], BF, isOutput=False)
    sinO = nc.declare_dram_parameter("sinO", [HALF, NQ], BF, isOutput=False)
    cosK = nc.declare_dram_parameter("cosK", [HALF, NQ], BF, isOutput=False)
    sinK = nc.declare_dram_parameter("sinK", [HALF, NQ], BF, isOutput=False)
    masks = nc.declare_dram_parameter("masks", [NSLOT, 2, KB, QB], BF, isOutput=False)
    out = nc.declare_dram_parameter("out", [NQ, D], F32, isOutput=True)

    ld = nc.sync       # streaming loads (q blocks, cos/sin, kt/vt, masks, wo)
    pf = nc.scalar     # prefetch loads (weight halves, first-step K/V)
    st = nc.gpsimd     # all stores + collectives

    with tile.TileContext(nc) as tc:
        from contextlib import ExitStack
        es = ExitStack()
        dram = es.enter_context(tc.tile_pool(name="dram", bufs=1, space="DRAM"))
        QT_d = dram.tile([D, NQ], BF, tag="QT_d")
        l_d = dram.tile([NSLOT, QB], F32, tag="l_d")
        kvK = [dram.tile([KTSZ], BF, tag=f"kvK{i}", name=f"kvK{i}")
               for i in range(4)]
        kvV = [dram.tile([KB * D], BF, tag=f"kvV{i}", name=f"kvV{i}")
               for i in range(4)]
        gK = [dram.tile([2, KTSZ], BF, tag=f"gK{i}", name=f"gK{i}")
              for i in range(4)]
        gV = [dram.tile([2, KB * D], BF, tag=f"gV{i}", name=f"gV{i}")
              for i in range(4)]

        def _ag(src_t, dst_t):
            nc.gpsimd.collective_compute(
                "AllGather",
                mybir.AluOpType.bypass,
                replica_groups=[[0, 1], [2, 3], [4, 5], [6, 7]],
                ins=[src_t[:].opt()],
                outs=[dst_t[:].opt()],
            )

        const = es.enter_context(tc.tile_pool(name="const", bufs=1))
        ones_t = const.tile([P, 1], BF, tag="ones")
        nc.vector.memset(ones_t, 1.0)

        # ---- persistent-ish pools ----
        wa_cm = tc.tile_pool(name="WA", bufs=1)                          # left
        wa = wa_cm.__enter__()
        wb_cm = tc.tile_pool(name="WB", bufs=1, side="right")
        wb = wb_cm.__enter__()

        # shared projection-phase working pools (left)
        qio_cm = tc.tile_pool(name="qio", bufs=2)
        qio = qio_cm.__enter__()
        csio_cm = tc.tile_pool(name="csio", bufs=2)
        csio = csio_cm.__enter__()

        def open_kotmp():
            ko_cm = tc.tile_pool(name="ko", bufs=3)
            tmp_cm = tc.tile_pool(name="tmp", bufs=2)
            return ko_cm, ko_cm.__enter__(), tmp_cm, tmp_cm.__enter__()

        def _w(w1, w2, di, cols):
            return (w1 if di < 8 else w2)[:, di % 8, cols]

        def _q(qa, qb, di):
            return (qa if di < 8 else qb)[:, di % 8]

        def load_w_pair(pool, src, eng1, eng2, tag1="w1", tag2="w2"):
            w1 = pool.tile([P, 8, D], BF, tag=tag1)
            w2 = pool.tile([P, 8, D], BF, tag=tag2)
            v = src.rearrange("(c p) f -> p c f", p=P)
            for c in range(8):
                eng1.dma_start(w1[:, c], v[:, c])
            for c in range(8):
                eng2.dma_start(w2[:, c], v[:, c + 8])
            return w1, w2

        # ================= K projection =================
        # Wk halves split across both queues for fast startup; Wv prefetch
        # (right slot) and block-0 cos/sin go to the store queue, idle now.
        ko_cm, kop, tmp_cm, tmpp = open_kotmp()

        w1, w2 = load_w_pair(wa, Wk, ld, pf)
        wv1 = wb.tile([P, 8, D], BF, tag="wv1")
        wv2 = wb.tile([P, 8, D], BF, tag="wv2")
        vv_ = Wv.rearrange("(c p) f -> p c f", p=P)

        def rope_block(qsrc_cols, cos_cols, sin_cols, w1, w2, emit,
                       q_eng=ld, cs_eng=ld, issue_after_q=None):
            qa = qio.tile([P, 8, QB], BF, tag="qa")
            qb = qio.tile([P, 8, QB], BF, tag="qb")
            vq = qsrc_cols.rearrange("(c p) f -> p c f", p=P)
            for c in range(8):
                q_eng.dma_start(qa[:, c], vq[:, c])
            for c in range(8):
                q_eng.dma_start(qb[:, c], vq[:, c + 8])
            if issue_after_q is not None:
                issue_after_q()
            cos_t = csio.tile([P, 8, QB], BF, tag="cos")
            sin_t = csio.tile([P, 8, QB], BF, tag="sin")
            _dma_in(cs_eng, cos_t, cos_cols, 8)
            _dma_in(cs_eng, sin_t, sin_cols, 8)
            for j in range(8):
                psA = pps.tile([P, QB], F32, tag="psA")
                psB = pps.tile([P, QB], F32, tag="psB")
                for di in range(DI):
                    nc.tensor.matmul(
                        psA, _w(w1, w2, di, slice(j * P, (j + 1) * P)),
                        _q(qa, qb, di),
                        start=(di == 0), stop=(di == DI - 1))
                for di in range(DI):
                    nc.tensor.matmul(
                        psB, _w(w1, w2, di, slice((j + 8) * P, (j + 9) * P)),
                        _q(qa, qb, di),
                        start=(di == 0), stop=(di == DI - 1))
                t1 = tmpp.tile([P, QB], F32, tag="t1")
                t2 = tmpp.tile([P, QB], F32, tag="t2")
                nc.vector.tensor_tensor(t1, psA, cos_t[:, j],
                                        mybir.AluOpType.mult)
                nc.vector.tensor_tensor(t2, psB, sin_t[:, j],
                                        mybir.AluOpType.mult)
                emit(j, t1, t2, True)
                nc.vector.tensor_tensor(t1, psA, sin_t[:, j],
                                        mybir.AluOpType.mult)
                nc.vector.tensor_tensor(t2, psB, cos_t[:, j],
                                        mybir.AluOpType.mult)
                emit(j + 8, t1, t2, False)

        with tc.tile_pool(name="pps", bufs=3, space="PSUM") as pps:
            for kb in range(4):
                def issue_wv():
                    # issue after block 1's loads so it doesn't steal startup
                    # bandwidth; transfers during K blocks 1-3, used in V.
                    # On the sync queue: the Act HWDGE queue posts completion
                    # semaphores so late that anything gating on its ring
                    # stalls for tens of us.
                    if kb == 1:
                        for c in range(8):
                            ld.dma_start(wv1[:, c], vv_[:, c])
                        for c in range(8):
                            ld.dma_start(wv2[:, c], vv_[:, c + 8])

                kv_view = kvK[kb][:].rearrange("(c p f) -> p c f", p=P, f=KB)

                def emit_k(ch, t1, t2, is_sub, kv_view=kv_view):
                    ko = kop.tile([P, QB], BF, tag="ko1" if is_sub else "ko2")
                    nc.vector.tensor_tensor(
                        ko, t1, t2,
                        mybir.AluOpType.subtract if is_sub
                        else mybir.AluOpType.add)
                    st.dma_start(kv_view[:, ch], ko)

                sl = slice(kb * KB, (kb + 1) * KB)
                rope_block(qT_keys[:, sl], cosK[:, sl], sinK[:, sl],
                           w1, w2, emit_k,
                           q_eng=(st if kb == 0 else ld),
                           cs_eng=(st if kb == 0 else ld),
                           issue_after_q=issue_wv)
                _ag(kvK[kb], gK[kb])

        # Wq prefetch into WA (waits on Wk's last reader; rides the store
        # queue behind the K AllGathers — transfers early in V, used in Q)
        wq1, wq2 = load_w_pair(wa, Wq, st, st)

        # ================= V projection =================
        tmp_cm.__exit__(None, None, None)
        ko_cm.__exit__(None, None, None)
        with (
            tc.tile_pool(name="vo", bufs=3) as vop,
            tc.tile_pool(name="vps", bufs=8, space="PSUM") as vps,
        ):
            # load q blocks one iteration ahead so the data-dependent vv
            # stores (same sync queue) never head-of-line block them
            def load_vq(kb):
                qa = qio.tile([P, 8, QB], BF, tag="qa", name=f"vqa{kb}")
                qb = qio.tile([P, 8, QB], BF, tag="qb", name=f"vqb{kb}")
                vq = qT_keys[:, kb * KB:(kb + 1) * KB].rearrange(
                    "(c p) f -> p c f", p=P)
                for c in range(8):
                    ld.dma_start(qa[:, c], vq[:, c])
                for c in range(8):
                    ld.dma_start(qb[:, c], vq[:, c + 8])
                return qa, qb

            nxt = load_vq(0)
            for kb in range(4):
                qa, qb = nxt
                if kb < 3:
                    nxt = load_vq(kb + 1)
                vv = kvV[kb][:].rearrange("(s d) -> s d", d=D)
                for ss in range(4):
                    vo = vop.tile([P, D], BF, tag="vo")
                    for dob in range(4):
                        ps = vps.tile([P, QB], F32, tag="vps")
                        for di in range(DI):
                            nc.tensor.matmul(
                                ps, _q(qa, qb, di)[:, ss * P:(ss + 1) * P],
                                _w(wv1, wv2, di,
                                   slice(dob * QB, (dob + 1) * QB)),
                                start=(di == 0), stop=(di == DI - 1))
                        nc.any.tensor_copy(vo[:, dob * QB:(dob + 1) * QB], ps)
                    ld.dma_start(vv[ss * P:(ss + 1) * P, :], vo)
                _ag(kvV[kb], gV[kb])

        # ================= Q projection =================
        # WB (right) closes; the prefetch pool takes its place holding slot
        # 3's Q (written directly by the epilogue) and the first attention
        # step's K/V tiles (DMA'd during Q-proj).
        wb_cm.__exit__(None, None, None)
        pre_cm = tc.tile_pool(name="prefetch", bufs=1, side="right")
        prep = pre_cm.__enter__()
        q3 = prep.tile([P, DI, QB], BF, tag="q3")
        kt0 = prep.tile([P, DI, KB], BF, tag="kt0")
        vt0 = prep.tile([P, 4, D], BF, tag="vt0")
        ag0, gi0 = AGIDX[0]

        ko_cm, kop, tmp_cm, tmpp = open_kotmp()
        with tc.tile_pool(name="pps", bufs=3, space="PSUM") as pps:
            for sb in SLOT_ORDER:
                if sb == 2:
                    # first attention step's K/V: issue after slot 3's own
                    # loads, transfers during the rest of Q-proj (sync queue)
                    _dma_in(ld, kt0,
                            gK[ag0][gi0].rearrange("(d s) -> d s", s=KB), DI)
                    _dma_in(ld, vt0,
                            gV[ag0][gi0].rearrange("(s d) -> s d", d=D), 4)
                sl = slice(sb * QB, (sb + 1) * QB)
                if sb == 3:
                    def emit_q(ch, t1, t2, is_sub):
                        nc.vector.tensor_tensor(
                            q3[:, ch], t1, t2,
                            mybir.AluOpType.subtract if is_sub
                            else mybir.AluOpType.add)
                else:
                    qv = QT_d[:, sl].rearrange("(c p) f -> p c f", p=P)

                    def emit_q(ch, t1, t2, is_sub, qv=qv):
                        ko = kop.tile([P, QB], BF,
                                      tag="ko1" if is_sub else "ko2")
                        nc.vector.tensor_tensor(
                            ko, t1, t2,
                            mybir.AluOpType.subtract if is_sub
                            else mybir.AluOpType.add)
                        st.dma_start(qv[:, ch], ko)

                rope_block(qT_own[:, sl], cosO[:, sl], sinO[:, sl],
                           wq1, wq2, emit_q)
        tmp_cm.__exit__(None, None, None)
        ko_cm.__exit__(None, None, None)
        csio_cm.__exit__(None, None, None)
        qio_cm.__exit__(None, None, None)
        wa_cm.__exit__(None, None, None)

        # ================= attention + output projection =================
        with (
            tc.tile_pool(name="qslot", bufs=1) as qslot,
            tc.tile_pool(name="kio", bufs=2) as kio,
            tc.tile_pool(name="vio", bufs=2) as vio,
            tc.tile_pool(name="pt", bufs=2) as ptpool,
            tc.tile_pool(name="mio", bufs=1) as mio,
            tc.tile_pool(name="ot", bufs=1) as otpool,
            tc.tile_pool(name="wo", bufs=2) as wopool,
            tc.tile_pool(name="fo", bufs=2) as fopool,
            tc.tile_pool(name="small", bufs=2) as small,
            tc.tile_pool(name="stps", bufs=2, space="PSUM") as stps,
            tc.tile_pool(name="pvps", bufs=3, space="PSUM") as pvps,
            tc.tile_pool(name="fps", bufs=2, space="PSUM") as fps,
            tc.tile_pool(name="lps", bufs=1, space="PSUM") as lps,
        ):
            for j in SLOT_ORDER:
                t = TSTEPS[j]
                if j == 3:
                    q_t = q3
                else:
                    q_t = qslot.tile([P, DI, QB], BF, tag="qs")
                    _dma_in(ld, q_t, QT_d[:, j * QB:(j + 1) * QB], DI)
                m_t = mio.tile([P, 2, 4, QB], BF, tag="mask")
                for sx in range(2):
                    _dma_in(ld, m_t[:, sx], masks[j, sx], 4)
                # per-chunk accumulator tiles: O-proj matmuls depend only on
                # the chunks they read, not on every PV add of the slot
                ot = [otpool.tile([P, QB], BF, tag=f"ot{do}",
                                  name=f"ot{do}_{j}")
                      for do in range(NCH)]
                l_ps = lps.tile([1, QB], F32, tag="lps")
                wo_pre = []
                for s in range(t):
                    ag, idx = AGIDX[s]
                    if s == 1:
                        # prefetch the first two Wo column groups behind this
                        # step's K/V loads so O-proj never waits on them
                        for dob in range(2):
                            wo_t = wopool.tile([P, DI, QB], BF, tag="wo",
                                               name=f"wo{dob}_{j}")
                            _dma_in(ld, wo_t,
                                    Wo[:, dob * QB:(dob + 1) * QB], DI)
                            wo_pre.append(wo_t)
                    if j == 3 and s == 0:
                        kt, vt = kt0, vt0
                    else:
                        kt = kio.tile([P, DI, KB], BF, tag="kt")
                        _dma_in(ld, kt,
                                gK[ag][idx].rearrange("(d s) -> d s", s=KB),
                                DI)
                        vt = vio.tile([P, 4, D], BF, tag="vt")
                        _dma_in(ld, vt,
                                gV[ag][idx].rearrange("(s d) -> s d", d=D), 4)
                    pt = ptpool.tile([P, 4, QB], BF, tag="pt")
                    final = (s == t - 1)
                    masked = s >= t - 2
                    for kc in range(4):
                        off = kc * P if final else 0
                        stile = stps.tile([P, QB], F32, tag="st")
                        for di in range(DI):
                            nc.tensor.matmul(
                                stile[:, off:],
                                kt[:, di, kc * P:(kc + 1) * P],
                                q_t[:, di, off:],
                                start=(di == 0), stop=(di == DI - 1))
                        if masked:
                            nc.vector.tensor_tensor(
                                stile[:, off:], stile[:, off:],
                                m_t[:, s - (t - 2), kc, off:],
                                mybir.AluOpType.add)
                        nc.scalar.activation(
                            pt[:, kc, off:], stile[:, off:],
                            mybir.ActivationFunctionType.Exp)
                        nc.tensor.matmul(
                            l_ps[:, off:], ones_t, pt[:, kc, off:],
                            start=(s == 0 and kc == 0),
                            stop=(final and kc == 3),
                            skip_group_check=True)
                    for do in range(NCH):
                        pv = pvps.tile([P, QB], F32, tag="pv")
                        for kc in range(4):
                            off = kc * P if final else 0
                            nc.tensor.matmul(
                                pv[:, off:],
                                vt[:, kc, do * P:(do + 1) * P],
                                pt[:, kc, off:],
                                start=(kc == 0), stop=(kc == 3),
                                skip_group_check=final)
                        if s == 0:
                            nc.vector.tensor_copy(ot[do], pv)
                        else:
                            nc.vector.tensor_tensor(
                                ot[do], ot[do], pv,
                                mybir.AluOpType.add)
                # O projection: bounce l through DRAM to transpose
                # [1, 512] -> per-partition columns [P, 4] (store queue).
                l_sb = small.tile([1, QB], F32, tag="lsb")
                nc.any.tensor_copy(l_sb, l_ps)
                st.dma_start(l_d[j:j + 1, :], l_sb)
                lcols = small.tile([P, 4], F32, tag="lcols")
                st.dma_start(lcols, l_d[j].rearrange("(qs p) -> p qs", p=P))
                inv_l = small.tile([P, 4], F32, tag="invl")
                nc.vector.reciprocal(inv_l, lcols)
                for dob in range(4):
                    if dob < 2:
                        wo_t = wo_pre[dob]
                    else:
                        wo_t = wopool.tile([P, DI, QB], BF, tag="wo")
                        _dma_in(ld, wo_t, Wo[:, dob * QB:(dob + 1) * QB], DI)
                    for qs in range(4):
                        f_ps = fps.tile([P, QB], F32, tag="fps")
                        for di in range(DI):
                            nc.tensor.matmul(
                                f_ps, ot[di][:, qs * P:(qs + 1) * P],
                                wo_t[:, di, :],
                                start=(di == 0), stop=(di == DI - 1))
                        fo = fopool.tile([P, QB], F32, tag="fo")
                        nc.vector.tensor_scalar_mul(
                            fo, f_ps, inv_l[:, qs:qs + 1])
                        st.dma_start(
                            out[j * QB + qs * P: j * QB + (qs + 1) * P,
                                dob * QB:(dob + 1) * QB], fo)
        pre_cm.__exit__(None, None, None)
        es.close()
    return nc


_NC_CACHE = None


def _get_nc():
    global _NC_CACHE
    if _NC_CACHE is None:
        _NC_CACHE = _build()
    return _NC_CACHE


def _host_prep(q, W_q, W_k, W_v, W_o):
    perm = np.concatenate([np.arange(0, D, 2), np.arange(1, D, 2)])
    scale = 1.0 / math.sqrt(D)
    Wq_p = np.ascontiguousarray((W_q * scale)[:, perm]).astype(bf16)
    Wk_p = np.ascontiguousarray(W_k[:, perm]).astype(bf16)
    Wv_p = W_v.astype(bf16)
    Wo_p = W_o.astype(bf16)
    inv_freq = 1.0 / (ROPE_BASE ** (np.arange(0, D, 2, dtype=np.float64) / D))
    ang = np.arange(S, dtype=np.float64)[:, None] * inv_freq[None, :]
    cosT = np.ascontiguousarray(np.cos(ang).T).astype(bf16)   # (HALF, S)
    sinT = np.ascontiguousarray(np.sin(ang).T).astype(bf16)
    return Wq_p, Wk_p, Wv_p, Wo_p, cosT, sinT


def _make_masks(blocks):
    m = np.zeros((NSLOT, 2, KB, QB), dtype=np.float32)
    k_idx = np.arange(KB)[:, None]
    q_idx = np.arange(QB)[None, :]
    tri = np.where(k_idx <= q_idx, 0.0, NEG).astype(np.float32)
    for j, blk in enumerate(blocks):
        t = TSTEPS[j]
        limit = blk + 1
        for sidx, s in enumerate([t - 2, t - 1]):
            if s == limit - 1:
                m[j, sidx] = tri
            elif s >= limit:
                m[j, sidx] = NEG
    return m.astype(bf16)


def run(inputs, trace=False):
    q = np.asarray(inputs["q"], dtype=np.float32)
    W_q = np.asarray(inputs["W_q"], dtype=np.float32)
    W_k = np.asarray(inputs["W_k"], dtype=np.float32)
    W_v = np.asarray(inputs["W_v"], dtype=np.float32)
    W_o = np.asarray(inputs["W_o"], dtype=np.float32)

    Wq_p, Wk_p, Wv_p, Wo_p, cosT, sinT = _host_prep(q, W_q, W_k, W_v, W_o)

    in_maps = []
    core_blocks = []
    for c in range(8):
        b = c // 2
        blocks = BLOCKS_EVEN if c % 2 == 0 else BLOCKS_ODD
        keys = KEYS_EVEN if c % 2 == 0 else KEYS_ODD
        core_blocks.append((b, blocks))
        qTb = np.ascontiguousarray(q[b].T).astype(bf16)       # (D, S)
        own_cols = np.concatenate(
            [np.arange(blk * QB, (blk + 1) * QB) for blk in blocks])
        key_cols = np.concatenate(
            [np.arange(blk * QB, (blk + 1) * QB) for blk in keys])
        in_maps.append({
            "qT_own": np.ascontiguousarray(qTb[:, own_cols]),
            "qT_keys": np.ascontiguousarray(qTb[:, key_cols]),
            "Wq": Wq_p, "Wk": Wk_p, "Wv": Wv_p, "Wo": Wo_p,
            "cosO": np.ascontiguousarray(cosT[:, own_cols]),
            "sinO": np.ascontiguousarray(sinT[:, own_cols]),
            "cosK": np.ascontiguousarray(cosT[:, key_cols]),
            "sinK": np.ascontiguousarray(sinT[:, key_cols]),
            "masks": _make_masks(blocks),
        })

    nc = _get_nc()
    res = run_bass_kernel_spmd(nc, in_maps, core_ids=list(range(8)),
                               trace=trace)

    out = np.zeros((B, S, D), dtype=np.float32)
    for c, (b, blocks) in enumerate(core_blocks):
        o = res.results[c]["out"]
        for j, blk in enumerate(blocks):
            out[b, blk * QB:(blk + 1) * QB] = o[j * QB:(j + 1) * QB]
    return out, res


def kernel(**inputs):
    return run(inputs, trace=False)[0]


# revision 32
# speedup vs baseline: 1.0700x; 1.0700x over previous
"""Trainium2 Bass kernel for single-head causal attention with RoPE.

Problem: B=4, S=4096, D=2048, H=1.
  out = softmax(causal(rope(q@Wq) @ rope(q@Wk)^T / sqrt(D))) @ (q@Wv) @ Wo

Sharding: 8 cores = 4 batches x 2 groups. Each core owns 4 of the batch's 8
512-row blocks ({7,4,3,0} even cores / {6,5,2,1} odd) — a causal-balanced
split: both cores see 18 real key-steps, padded to a uniform per-slot
schedule TSTEPS=[8,6,4,2] so all cores run one NEFF. K/V projections are
computed only for OWN blocks; the pair exchanges K/V per block-pair through
four 2-rank AllGathers that overlap later projection phases.

All matmuls bf16 (PSUM accumulates fp32). RoPE is reduced to half-rotation by
permuting Wq/Wk columns on the host; the score scale 1/sqrt(D) is folded into
Wq. Attention is transpose-free: scores are computed as S^T[k,q] per block,
softmax runs without max-subtraction, the denominator comes from per-qs
pt^T@ones matmuls accumulated directly in [P,4] orientation, and P^T feeds
the PV matmul directly.

Perf structure (vs the first working version):
- Two rotating weight slots (WA left / WB right): Wv prefetches during
  K-proj, Wq during V-proj, so phase transitions never stall on weight DMA.
- DMA queues split by role: sync=streaming loads, scalar=weight prefetch +
  first-attention-step prefetch, gpsimd=all stores + collectives. This avoids
  head-of-line blocking of latency-critical loads behind data-dependent
  stores.
- Slot 3's Q stays in SBUF (no DRAM round trip); first attention step's K/V
  tiles prefetch during Q-proj.
- The FINAL key-step of every slot is triangle-trimmed: per kc chunk only
  queries >= kc*128 are computed in QK/exp/l/PV (on even cores that step is
  the fully-masked pad step; trimming is a superset of its needs).
- ot accumulates in bf16 (no separate cast tile before the O projection).
"""

import json
import math

import ml_dtypes
import numpy as np

import concourse.bass as bass
import concourse.mybir as mybir
import concourse.tile as tile
from concourse.bass_utils import run_bass_kernel_spmd


def _split_multi_waits(bir_json_bytes):
    """Rewrite BIR so no instruction carries more than one semaphore wait.

    The walrus build in this environment rejects instructions with >1 sync
    wait. Extra waits are hoisted onto injected same-engine EventSemaphore
    instructions placed immediately before the instruction (engine program
    order makes them gate it)."""
    d = json.loads(bir_json_bytes)
    for fn in d["functions"]:
        for blk in fn["blocks"]:
            out = []
            for inst in blk["instructions"]:
                si = inst.get("sync_info") or {}
                ow = si.get("on_wait") or []
                if len(ow) > 1:
                    for i, w in enumerate(ow[:-1]):
                        out.append({
                            "debug": inst.get("debug"),
                            "engine": inst["engine"],
                            "ins": [],
                            "outs": [],
                            "name": f"{inst['name']}_sw{i}",
                            "opcode": "EventSemaphore",
                            "sync_info": {"on_update": [], "on_wait": [w]},
                        })
                    si["on_wait"] = [ow[-1]]
                out.append(inst)
            blk["instructions"] = out
    return json.dumps(d).encode()


def _install_split_waits():
    import concourse.bass_utils as bu
    if getattr(bu, "_split_waits_installed", False):
        return
    orig = bu.compile_bir_kernel

    def patched(bir_json, tmpdir, neff_name="file.neff"):
        return orig(_split_multi_waits(bir_json), tmpdir, neff_name)

    bu.compile_bir_kernel = patched
    bu._split_waits_installed = True
    import concourse.bass2jax as b2j
    if getattr(b2j, "compile_bir_kernel", None) is orig:
        b2j.compile_bir_kernel = patched


_install_split_waits()

BF = mybir.dt.bfloat16
F32 = mybir.dt.float32
bf16 = ml_dtypes.bfloat16

B, S, D = 4, 4096, 2048
HALF = D // 2
P = 128
QB = 512           # query block (one slot)
KB = 512           # key step
NSLOT = 4          # query blocks per core
NQ = NSLOT * QB    # 2048 own queries per core
DI = D // P        # 16 contraction chunks
NCH = D // P       # 16 output chunks
TSTEPS = [8, 6, 4, 2]   # padded key steps per slot (uniform across cores)
SLOT_ORDER = [3, 2, 1, 0]   # processing order: ascending step counts
BLOCKS_EVEN = [7, 4, 3, 0]
BLOCKS_ODD = [6, 5, 2, 1]
KEYS_EVEN = [0, 3, 4, 7]   # same blocks, ascending (AllGather pairing order)
KEYS_ODD = [1, 2, 5, 6]
# key block s of the batch lives at gathered[AGIDX[s][0]][AGIDX[s][1]]
AGIDX = [(0, 0), (0, 1), (1, 1), (1, 0), (2, 0), (2, 1), (3, 1), (3, 0)]
KTSZ = D * KB              # elements of one K^T block [D, 512]
ROPE_BASE = 10000.0
NEG = -1.0e30


def _dma_in(eng, dst, src_ap, n):
    """Per-chunk DMA load of a [P, n, F] tile from a "(c p) f" DRAM view."""
    v = src_ap.rearrange("(c p) f -> p c f", p=P)
    for c in range(n):
        eng.dma_start(dst[:, c], v[:, c])


def _build():
    nc = bass.Bass(num_devices=8)

    qT_own = nc.declare_dram_parameter("qT_own", [D, NQ], BF, isOutput=False)
    qT_keys = nc.declare_dram_parameter("qT_keys", [D, NQ], BF, isOutput=False)
    Wq = nc.declare_dram_parameter("Wq", [D, D], BF, isOutput=False)
    Wk = nc.declare_dram_parameter("Wk", [D, D], BF, isOutput=False)
    Wv = nc.declare_dram_parameter("Wv", [D, D], BF, isOutput=False)
    Wo = nc.declare_dram_parameter("Wo", [D, D], BF, isOutput=False)
    cosO = nc.declare_dram_parameter("cosO", [HALF, NQ], BF, isOutput=False)
    sinO = nc.declare_dram_parameter("sinO", [HALF, NQ], BF, isOutput=False)
    cosK = nc.declare_dram_parameter("cosK", [HALF, NQ], BF, isOutput=False)
    sinK = nc.declare_dram_parameter("sinK", [HALF, NQ], BF, isOutput=False)
    masks = nc.declare_dram_parameter("masks", [NSLOT, 2, KB, QB], BF, isOutput=False)
    out = nc.declare_dram_parameter("out", [NQ, D], F32, isOutput=True)

    ld = nc.sync       # streaming loads (q blocks, cos/sin, kt/vt, masks, wo)
    pf = nc.scalar     # prefetch loads (weight halves, first-step K/V)
    st = nc.gpsimd     # all stores + collectives

    with tile.TileContext(nc) as tc:
        from contextlib import ExitStack
        es = ExitStack()
        dram = es.enter_context(tc.tile_pool(name="dram", bufs=1, space="DRAM"))
        QT_d = dram.tile([D, NQ], BF, tag="QT_d")
        l_d = dram.tile([NSLOT, QB], F32, tag="l_d")
        kvK = [dram.tile([KTSZ], BF, tag=f"kvK{i}", name=f"kvK{i}")
               for i in range(4)]
        kvV = [dram.tile([KB * D], BF, tag=f"kvV{i}", name=f"kvV{i}")
               for i in range(4)]
        gK = [dram.tile([2, KTSZ], BF, tag=f"gK{i}", name=f"gK{i}")
              for i in range(4)]
        gV = [dram.tile([2, KB * D], BF, tag=f"gV{i}", name=f"gV{i}")
              for i in range(4)]

        def _ag(src_t, dst_t):
            nc.gpsimd.collective_compute(
                "AllGather",
                mybir.AluOpType.bypass,
                replica_groups=[[0, 1], [2, 3], [4, 5], [6, 7]],
                ins=[src_t[:].opt()],
                outs=[dst_t[:].opt()],
            )

        const = es.enter_context(tc.tile_pool(name="const", bufs=1))
        ones_t = const.tile([P, 1], BF, tag="ones")
        nc.vector.memset(ones_t, 1.0)

        # ---- persistent-ish pools ----
        wa_cm = tc.tile_pool(name="WA", bufs=1)                          # left
        wa = wa_cm.__enter__()
        wb_cm = tc.tile_pool(name="WB", bufs=1, side="right")
        wb = wb_cm.__enter__()

        # shared projection-phase working pools (left)
        qio_cm = tc.tile_pool(name="qio", bufs=2)
        qio = qio_cm.__enter__()
        csio_cm = tc.tile_pool(name="csio", bufs=2)
        csio = csio_cm.__enter__()

        def open_kotmp():
            ko_cm = tc.tile_pool(name="ko", bufs=3)
            tmp_cm = tc.tile_pool(name="tmp", bufs=2)
            return ko_cm, ko_cm.__enter__(), tmp_cm, tmp_cm.__enter__()

        def _w(w1, w2, di, cols):
            return (w1 if di < 8 else w2)[:, di % 8, cols]

        def _q(qa, qb, di):
            return (qa if di < 8 else qb)[:, di % 8]

        def load_w_pair(pool, src, eng1, eng2, tag1="w1", tag2="w2"):
            w1 = pool.tile([P, 8, D], BF, tag=tag1)
            w2 = pool.tile([P, 8, D], BF, tag=tag2)
            v = src.rearrange("(c p) f -> p c f", p=P)
            for c in range(8):
                eng1.dma_start(w1[:, c], v[:, c])
            for c in range(8):
                eng2.dma_start(w2[:, c], v[:, c + 8])
            return w1, w2

        # ================= K projection =================
        # Wk halves split across both queues for fast startup; Wv prefetch
        # (right slot) and block-0 cos/sin go to the store queue, idle now.
        ko_cm, kop, tmp_cm, tmpp = open_kotmp()

        w1, w2 = load_w_pair(wa, Wk, ld, pf)
        wv1 = wb.tile([P, 8, D], BF, tag="wv1")
        wv2 = wb.tile([P, 8, D], BF, tag="wv2")
        vv_ = Wv.rearrange("(c p) f -> p c f", p=P)

        def rope_block(qsrc_cols, cos_cols, sin_cols, w1, w2, emit,
                       q_eng=ld, cs_eng=ld, issue_after_q=None):
            qa = qio.tile([P, 8, QB], BF, tag="qa")
            qb = qio.tile([P, 8, QB], BF, tag="qb")
            vq = qsrc_cols.rearrange("(c p) f -> p c f", p=P)
            for c in range(8):
                q_eng.dma_start(qa[:, c], vq[:, c])
            for c in range(8):
                q_eng.dma_start(qb[:, c], vq[:, c + 8])
            if issue_after_q is not None:
                issue_after_q()
            cos_t = csio.tile([P, 8, QB], BF, tag="cos")
            sin_t = csio.tile([P, 8, QB], BF, tag="sin")
            _dma_in(cs_eng, cos_t, cos_cols, 8)
            _dma_in(cs_eng, sin_t, sin_cols, 8)
            for j in range(8):
                psA = pps.tile([P, QB], F32, tag="psA")
                psB = pps.tile([P, QB], F32, tag="psB")
                for di in range(DI):
                    nc.tensor.matmul(
                        psA, _w(w1, w2, di, slice(j * P, (j + 1) * P)),
                        _q(qa, qb, di),
                        start=(di == 0), stop=(di == DI - 1))
                for di in range(DI):
                    nc.tensor.matmul(
                        psB, _w(w1, w2, di, slice((j + 8) * P, (j + 9) * P)),
                        _q(qa, qb, di),
                        start=(di == 0), stop=(di == DI - 1))
                t1 = tmpp.tile([P, QB], F32, tag="t1")
                t2 = tmpp.tile([P, QB], F32, tag="t2")
                nc.vector.tensor_tensor(t1, psA, cos_t[:, j],
                                        mybir.AluOpType.mult)
                nc.vector.tensor_tensor(t2, psB, sin_t[:, j],
                                        mybir.AluOpType.mult)
                emit(j, t1, t2, True)
                nc.vector.tensor_tensor(t1, psA, sin_t[:, j],
                                        mybir.AluOpType.mult)
                nc.vector.tensor_tensor(t2, psB, cos_t[:, j],
                                        mybir.AluOpType.mult)
                emit(j + 8, t1, t2, False)

        with tc.tile_pool(name="pps", bufs=3, space="PSUM") as pps:
            for kb in range(4):
                def issue_wv():
                    # issue after block 1's loads so it doesn't steal startup
                    # bandwidth; transfers during K blocks 1-3, used in V.
                    # On the sync queue: the Act HWDGE queue posts completion
                    # semaphores so late that anything gating on its ring
                    # stalls for tens of us.
                    if kb == 1:
                        for c in range(8):
                            ld.dma_start(wv1[:, c], vv_[:, c])
                        for c in range(8):
                            ld.dma_start(wv2[:, c], vv_[:, c + 8])

                kv_view = kvK[kb][:].rearrange("(c p f) -> p c f", p=P, f=KB)

                def emit_k(ch, t1, t2, is_sub, kv_view=kv_view):
                    ko = kop.tile([P, QB], BF, tag="ko1" if is_sub else "ko2")
                    nc.vector.tensor_tensor(
                        ko, t1, t2,
                        mybir.AluOpType.subtract if is_sub
                        else mybir.AluOpType.add)
                    st.dma_start(kv_view[:, ch], ko)

                sl = slice(kb * KB, (kb + 1) * KB)
                rope_block(qT_keys[:, sl], cosK[:, sl], sinK[:, sl],
                           w1, w2, emit_k,
                           q_eng=(st if kb == 0 else ld),
                           cs_eng=(st if kb == 0 else ld),
                           issue_after_q=issue_wv)
                _ag(kvK[kb], gK[kb])

        # Wq prefetch into WA (waits on Wk's last reader; rides the store
        # queue behind the K AllGathers — transfers early in V, used in Q)
        wq1, wq2 = load_w_pair(wa, Wq, st, st)

        # ================= V projection =================
        tmp_cm.__exit__(None, None, None)
        ko_cm.__exit__(None, None, None)
        with (
            tc.tile_pool(name="vo", bufs=3) as vop,
            tc.tile_pool(name="vps", bufs=8, space="PSUM") as vps,
        ):
            # load q blocks one iteration ahead so the data-dependent vv
            # stores (same sync queue) never head-of-line block them
            def load_vq(kb):
                qa = qio.tile([P, 8, QB], BF, tag="qa", name=f"vqa{kb}")
                qb = qio.tile([P, 8, QB], BF, tag="qb", name=f"vqb{kb}")
                vq = qT_keys[:, kb * KB:(kb + 1) * KB].rearrange(
                    "(c p) f -> p c f", p=P)
                for c in range(8):
                    ld.dma_start(qa[:, c], vq[:, c])
                for c in range(8):
                    ld.dma_start(qb[:, c], vq[:, c + 8])
                return qa, qb

            nxt = load_vq(0)
            for kb in range(4):
                qa, qb = nxt
                if kb < 3:
                    nxt = load_vq(kb + 1)
                vv = kvV[kb][:].rearrange("(s d) -> s d", d=D)
                for ss in range(4):
                    vo = vop.tile([P, D], BF, tag="vo")
                    for dob in range(4):
                        ps = vps.tile([P, QB], F32, tag="vps")
                        for di in range(DI):
                            nc.tensor.matmul(
                                ps, _q(qa, qb, di)[:, ss * P:(ss + 1) * P],
                                _w(wv1, wv2, di,
                                   slice(dob * QB, (dob + 1) * QB)),
                                start=(di == 0), stop=(di == DI - 1))
                        nc.any.tensor_copy(vo[:, dob * QB:(dob + 1) * QB], ps)
                    ld.dma_start(vv[ss * P:(ss + 1) * P, :], vo)
                _ag(kvV[kb], gV[kb])

        # ================= Q projection =================
        # WB (right) closes; the prefetch pool takes its place holding slot
        # 3's Q (written directly by the epilogue) and the first attention
        # step's K/V tiles (DMA'd during Q-proj).
        wb_cm.__exit__(None, None, None)
        pre_cm = tc.tile_pool(name="prefetch", bufs=1, side="right")
        prep = pre_cm.__enter__()
        q3 = prep.tile([P, DI, QB], BF, tag="q3")
        kt0 = prep.tile([P, DI, KB], BF, tag="kt0")
        vt0 = prep.tile([P, 4, D], BF, tag="vt0")
        ag0, gi0 = AGIDX[0]

        ko_cm, kop, tmp_cm, tmpp = open_kotmp()
        with tc.tile_pool(name="pps", bufs=3, space="PSUM") as pps:
            for sb in SLOT_ORDER:
                if sb == 2:
                    # first attention step's K/V: issue after slot 3's own
                    # loads, transfers during the rest of Q-proj (sync queue)
                    _dma_in(ld, kt0,
                            gK[ag0][gi0].rearrange("(d s) -> d s", s=KB), DI)
                    _dma_in(ld, vt0,
                            gV[ag0][gi0].rearrange("(s d) -> s d", d=D), 4)
                sl = slice(sb * QB, (sb + 1) * QB)
                if sb == 3:
                    def emit_q(ch, t1, t2, is_sub):
                        nc.vector.tensor_tensor(
                            q3[:, ch], t1, t2,
                            mybir.AluOpType.subtract if is_sub
                            else mybir.AluOpType.add)
                else:
                    qv = QT_d[:, sl].rearrange("(c p) f -> p c f", p=P)

                    def emit_q(ch, t1, t2, is_sub, qv=qv):
                        ko = kop.tile([P, QB], BF,
                                      tag="ko1" if is_sub else "ko2")
                        nc.vector.tensor_tensor(
                            ko, t1, t2,
                            mybir.AluOpType.subtract if is_sub
                            else mybir.AluOpType.add)
                        st.dma_start(qv[:, ch], ko)

                rope_block(qT_own[:, sl], cosO[:, sl], sinO[:, sl],
                           wq1, wq2, emit_q)
        tmp_cm.__exit__(None, None, None)
        ko_cm.__exit__(None, None, None)
        csio_cm.__exit__(None, None, None)
        qio_cm.__exit__(None, None, None)
        wa_cm.__exit__(None, None, None)

        # ================= attention + output projection =================
        with (
            tc.tile_pool(name="qslot", bufs=1) as qslot,
            tc.tile_pool(name="kio", bufs=2) as kio,
            tc.tile_pool(name="vio", bufs=2) as vio,
            tc.tile_pool(name="pt", bufs=2) as ptpool,
            tc.tile_pool(name="mio", bufs=1) as mio,
            tc.tile_pool(name="ot", bufs=1) as otpool,
            tc.tile_pool(name="wo", bufs=2) as wopool,
            tc.tile_pool(name="fo", bufs=2) as fopool,
            tc.tile_pool(name="small", bufs=2) as small,
            tc.tile_pool(name="stps", bufs=2, space="PSUM") as stps,
            tc.tile_pool(name="pvps", bufs=3, space="PSUM") as pvps,
            tc.tile_pool(name="fps", bufs=2, space="PSUM") as fps,
            tc.tile_pool(name="lps", bufs=1, space="PSUM") as lps,
        ):
            for j in SLOT_ORDER:
                t = TSTEPS[j]
                if j == 3:
                    q_t = q3
                else:
                    q_t = qslot.tile([P, DI, QB], BF, tag="qs")
                    _dma_in(ld, q_t, QT_d[:, j * QB:(j + 1) * QB], DI)
                m_t = mio.tile([P, 2, 4, QB], BF, tag="mask")
                for sx in range(2):
                    _dma_in(ld, m_t[:, sx], masks[j, sx], 4)
                # per-chunk accumulator tiles: O-proj matmuls depend only on
                # the chunks they read, not on every PV add of the slot
                ot = [otpool.tile([P, QB], BF, tag=f"ot{do}",
                                  name=f"ot{do}_{j}")
                      for do in range(NCH)]
                l_ps = lps.tile([1, QB], F32, tag="lps")
                wo_pre = []
                for s in range(t):
                    ag, idx = AGIDX[s]
                    if s == t - 1:
                        # prefetch the first two Wo column groups behind the
                        # final step's K/V loads (nothing else needs the
                        # queue until the next slot's q) so O-proj never
                        # waits on them
                        for dob in range(2):
                            wo_t = wopool.tile([P, DI, QB], BF, tag="wo",
                                               name=f"wo{dob}_{j}")
                            _dma_in(ld, wo_t,
                                    Wo[:, dob * QB:(dob + 1) * QB], DI)
                            wo_pre.append(wo_t)
                    if j == 3 and s == 0:
                        kt, vt = kt0, vt0
                    else:
                        kt = kio.tile([P, DI, KB], BF, tag="kt")
                        _dma_in(ld, kt,
                                gK[ag][idx].rearrange("(d s) -> d s", s=KB),
                                DI)
                        vt = vio.tile([P, 4, D], BF, tag="vt")
                        _dma_in(ld, vt,
                                gV[ag][idx].rearrange("(s d) -> s d", d=D), 4)
                    pt = ptpool.tile([P, 4, QB], BF, tag="pt")
                    final = (s == t - 1)
                    masked = s >= t - 2
                    for kc in range(4):
                        off = kc * P if final else 0
                        stile = stps.tile([P, QB], F32, tag="st")
                        for di in range(DI):
                            nc.tensor.matmul(
                                stile[:, off:],
                                kt[:, di, kc * P:(kc + 1) * P],
                                q_t[:, di, off:],
                                start=(di == 0), stop=(di == DI - 1))
                        if masked:
                            nc.vector.tensor_tensor(
                                stile[:, off:], stile[:, off:],
                                m_t[:, s - (t - 2), kc, off:],
                                mybir.AluOpType.add)
                        nc.scalar.activation(
                            pt[:, kc, off:], stile[:, off:],
                            mybir.ActivationFunctionType.Exp)
                        nc.tensor.matmul(
                            l_ps[:, off:], ones_t, pt[:, kc, off:],
                            start=(s == 0 and kc == 0),
                            stop=(final and kc == 3),
                            skip_group_check=True)
                    for do in range(NCH):
                        pv = pvps.tile([P, QB], F32, tag="pv")
                        for kc in range(4):
                            off = kc * P if final else 0
                            nc.tensor.matmul(
                                pv[:, off:],
                                vt[:, kc, do * P:(do + 1) * P],
                                pt[:, kc, off:],
                                start=(kc == 0), stop=(kc == 3),
                                skip_group_check=final)
                        if s == 0:
                            nc.vector.tensor_copy(ot[do], pv)
                        else:
                            nc.vector.tensor_tensor(
                                ot[do], ot[do], pv,
                                mybir.AluOpType.add)
                # O projection: bounce l through DRAM to transpose
                # [1, 512] -> per-partition columns [P, 4] (store queue).
                l_sb = small.tile([1, QB], F32, tag="lsb")
                nc.any.tensor_copy(l_sb, l_ps)
                st.dma_start(l_d[j:j + 1, :], l_sb)
                lcols = small.tile([P, 4], F32, tag="lcols")
                st.dma_start(lcols, l_d[j].rearrange("(qs p) -> p qs", p=P))
                inv_l = small.tile([P, 4], F32, tag="invl")
                nc.vector.reciprocal(inv_l, lcols)
                for dob in range(4):
                    if dob < 2:
                        wo_t = wo_pre[dob]
                    else:
                        wo_t = wopool.tile([P, DI, QB], BF, tag="wo")
                        _dma_in(ld, wo_t, Wo[:, dob * QB:(dob + 1) * QB], DI)
                    for qs in range(4):
                        f_ps = fps.tile([P, QB], F32, tag="fps")
                        for di in range(DI):
                            nc.tensor.matmul(
                                f_ps, ot[di][:, qs * P:(qs + 1) * P],
                                wo_t[:, di, :],
                                start=(di == 0), stop=(di == DI - 1))
                        fo = fopool.tile([P, QB], F32, tag="fo")
                        nc.vector.tensor_scalar_mul(
                            fo, f_ps, inv_l[:, qs:qs + 1])
                        st.dma_start(
                            out[j * QB + qs * P: j * QB + (qs + 1) * P,
                                dob * QB:(dob + 1) * QB], fo)
        pre_cm.__exit__(None, None, None)
        es.close()
    return nc


_NC_CACHE = None


def _get_nc():
    global _NC_CACHE
    if _NC_CACHE is None:
        _NC_CACHE = _build()
    return _NC_CACHE


def _host_prep(q, W_q, W_k, W_v, W_o):
    perm = np.concatenate([np.arange(0, D, 2), np.arange(1, D, 2)])
    scale = 1.0 / math.sqrt(D)
    Wq_p = np.ascontiguousarray((W_q * scale)[:, perm]).astype(bf16)
    Wk_p = np.ascontiguousarray(W_k[:, perm]).astype(bf16)
    Wv_p = W_v.astype(bf16)
    Wo_p = W_o.astype(bf16)
    inv_freq = 1.0 / (ROPE_BASE ** (np.arange(0, D, 2, dtype=np.float64) / D))
    ang = np.arange(S, dtype=np.float64)[:, None] * inv_freq[None, :]
    cosT = np.ascontiguousarray(np.cos(ang).T).astype(bf16)   # (HALF, S)
    sinT = np.ascontiguousarray(np.sin(ang).T).astype(bf16)
    return Wq_p, Wk_p, Wv_p, Wo_p, cosT, sinT


def _make_masks(blocks):
    m = np.zeros((NSLOT, 2, KB, QB), dtype=np.float32)
    k_idx = np.arange(KB)[:, None]
    q_idx = np.arange(QB)[None, :]
    tri = np.where(k_idx <= q_idx, 0.0, NEG).astype(np.float32)
    for j, blk in enumerate(blocks):
        t = TSTEPS[j]
        limit = blk + 1
        for sidx, s in enumerate([t - 2, t - 1]):
            if s == limit - 1:
                m[j, sidx] = tri
            elif s >= limit:
                m[j, sidx] = NEG
    return m.astype(bf16)


def run(inputs, trace=False):
    q = np.asarray(inputs["q"], dtype=np.float32)
    W_q = np.asarray(inputs["W_q"], dtype=np.float32)
    W_k = np.asarray(inputs["W_k"], dtype=np.float32)
    W_v = np.asarray(inputs["W_v"], dtype=np.float32)
    W_o = np.asarray(inputs["W_o"], dtype=np.float32)

    Wq_p, Wk_p, Wv_p, Wo_p, cosT, sinT = _host_prep(q, W_q, W_k, W_v, W_o)

    in_maps = []
    core_blocks = []
    for c in range(8):
        b = c // 2
        blocks = BLOCKS_EVEN if c % 2 == 0 else BLOCKS_ODD
        keys = KEYS_EVEN if c % 2 == 0 else KEYS_ODD
        core_blocks.append((b, blocks))
        qTb = np.ascontiguousarray(q[b].T).astype(bf16)       # (D, S)
        own_cols = np.concatenate(
            [np.arange(blk * QB, (blk + 1) * QB) for blk in blocks])
        key_cols = np.concatenate(
            [np.arange(blk * QB, (blk + 1) * QB) for blk in keys])
        in_maps.append({
            "qT_own": np.ascontiguousarray(qTb[:, own_cols]),
            "qT_keys": np.ascontiguousarray(qTb[:, key_cols]),
            "Wq": Wq_p, "Wk": Wk_p, "Wv": Wv_p, "Wo": Wo_p,
            "cosO": np.ascontiguousarray(cosT[:, own_cols]),
            "sinO": np.ascontiguousarray(sinT[:, own_cols]),
            "cosK": np.ascontiguousarray(cosT[:, key_cols]),
            "sinK": np.ascontiguousarray(sinT[:, key_cols]),
            "masks": _make_masks(blocks),
        })

    nc = _get_nc()
    res = run_bass_kernel_spmd(nc, in_maps, core_ids=list(range(8)),
                               trace=trace)

    out = np.zeros((B, S, D), dtype=np.float32)
    for c, (b, blocks) in enumerate(core_blocks):
        o = res.results[c]["out"]
        for j, blk in enumerate(blocks):
            out[b, blk * QB:(blk + 1) * QB] = o[j * QB:(j + 1) * QB]
    return out, res


def kernel(**inputs):
    return run(inputs, trace=False)[0]


# revision 33
# speedup vs baseline: 1.1539x; 1.0784x over previous
"""Trainium2 Bass kernel for single-head causal attention with RoPE.

Problem: B=4, S=4096, D=2048, H=1.
  out = softmax(causal(rope(q@Wq) @ rope(q@Wk)^T / sqrt(D))) @ (q@Wv) @ Wo

Sharding: 8 cores = 4 batches x 2 groups. Each core owns 4 of the batch's 8
512-row blocks ({7,4,3,0} even cores / {6,5,2,1} odd) — a causal-balanced
split: both cores see 18 real key-steps, padded to a uniform per-slot
schedule TSTEPS=[8,6,4,2] so all cores run one NEFF. K/V projections are
computed only for OWN blocks; the pair exchanges K/V per block-pair through
four 2-rank AllGathers that overlap later projection phases.

All matmuls bf16 (PSUM accumulates fp32). RoPE is reduced to half-rotation by
permuting Wq/Wk columns on the host; the score scale 1/sqrt(D) is folded into
Wq. Attention is transpose-free: scores are computed as S^T[k,q] per block,
softmax runs without max-subtraction, the denominator comes from per-qs
pt^T@ones matmuls accumulated directly in [P,4] orientation, and P^T feeds
the PV matmul directly.

Perf structure (vs the first working version):
- Two rotating weight slots (WA left / WB right): Wv prefetches during
  K-proj, Wq during V-proj, so phase transitions never stall on weight DMA.
- DMA queues split by role: sync=streaming loads, scalar=weight prefetch +
  first-attention-step prefetch, gpsimd=all stores + collectives. This avoids
  head-of-line blocking of latency-critical loads behind data-dependent
  stores.
- Slot 3's Q stays in SBUF (no DRAM round trip); first attention step's K/V
  tiles prefetch during Q-proj.
- The FINAL key-step of every slot is triangle-trimmed: per kc chunk only
  queries >= kc*128 are computed in QK/exp/l/PV (on even cores that step is
  the fully-masked pad step; trimming is a superset of its needs).
- ot accumulates in bf16 (no separate cast tile before the O projection).
"""

import json
import math

import ml_dtypes
import numpy as np

import concourse.bass as bass
import concourse.mybir as mybir
import concourse.tile as tile
from concourse.bass_utils import run_bass_kernel_spmd


def _split_multi_waits(bir_json_bytes):
    """Rewrite BIR so no instruction carries more than one semaphore wait.

    The walrus build in this environment rejects instructions with >1 sync
    wait. Extra waits are hoisted onto injected same-engine EventSemaphore
    instructions placed immediately before the instruction (engine program
    order makes them gate it)."""
    d = json.loads(bir_json_bytes)
    for fn in d["functions"]:
        for blk in fn["blocks"]:
            out = []
            for inst in blk["instructions"]:
                si = inst.get("sync_info") or {}
                ow = si.get("on_wait") or []
                if len(ow) > 1:
                    for i, w in enumerate(ow[:-1]):
                        out.append({
                            "debug": inst.get("debug"),
                            "engine": inst["engine"],
                            "ins": [],
                            "outs": [],
                            "name": f"{inst['name']}_sw{i}",
                            "opcode": "EventSemaphore",
                            "sync_info": {"on_update": [], "on_wait": [w]},
                        })
                    si["on_wait"] = [ow[-1]]
                out.append(inst)
            blk["instructions"] = out
    return json.dumps(d).encode()


def _install_split_waits():
    import concourse.bass_utils as bu
    if getattr(bu, "_split_waits_installed", False):
        return
    orig = bu.compile_bir_kernel

    def patched(bir_json, tmpdir, neff_name="file.neff"):
        return orig(_split_multi_waits(bir_json), tmpdir, neff_name)

    bu.compile_bir_kernel = patched
    bu._split_waits_installed = True
    import concourse.bass2jax as b2j
    if getattr(b2j, "compile_bir_kernel", None) is orig:
        b2j.compile_bir_kernel = patched


_install_split_waits()

BF = mybir.dt.bfloat16
F32 = mybir.dt.float32
bf16 = ml_dtypes.bfloat16

B, S, D = 4, 4096, 2048
HALF = D // 2
P = 128
QB = 512           # query block (one slot)
KB = 512           # key step
NSLOT = 4          # query blocks per core
NQ = NSLOT * QB    # 2048 own queries per core
DI = D // P        # 16 contraction chunks
NCH = D // P       # 16 output chunks
TSTEPS = [8, 6, 4, 2]   # padded key steps per slot (uniform across cores)
SLOT_ORDER = [3, 2, 1, 0]   # processing order: ascending step counts
BLOCKS_EVEN = [7, 4, 3, 0]
BLOCKS_ODD = [6, 5, 2, 1]
KEYS_EVEN = [0, 3, 4, 7]   # same blocks, ascending (AllGather pairing order)
KEYS_ODD = [1, 2, 5, 6]
# key block s of the batch lives at gathered[AGIDX[s][0]][AGIDX[s][1]]
AGIDX = [(0, 0), (0, 1), (1, 1), (1, 0), (2, 0), (2, 1), (3, 1), (3, 0)]
KTSZ = D * KB              # elements of one K^T block [D, 512]
ROPE_BASE = 10000.0
NEG = -1.0e30


def _dma_in(eng, dst, src_ap, n):
    """Per-chunk DMA load of a [P, n, F] tile from a "(c p) f" DRAM view."""
    v = src_ap.rearrange("(c p) f -> p c f", p=P)
    for c in range(n):
        eng.dma_start(dst[:, c], v[:, c])


def _build():
    nc = bass.Bass(num_devices=8)

    qT_own = nc.declare_dram_parameter("qT_own", [D, NQ], BF, isOutput=False)
    qT_keys = nc.declare_dram_parameter("qT_keys", [D, NQ], BF, isOutput=False)
    Wq = nc.declare_dram_parameter("Wq", [D, D], BF, isOutput=False)
    Wk = nc.declare_dram_parameter("Wk", [D, D], BF, isOutput=False)
    Wv = nc.declare_dram_parameter("Wv", [D, D], BF, isOutput=False)
    Wo = nc.declare_dram_parameter("Wo", [D, D], BF, isOutput=False)
    cosO = nc.declare_dram_parameter("cosO", [HALF, NQ], BF, isOutput=False)
    sinO = nc.declare_dram_parameter("sinO", [HALF, NQ], BF, isOutput=False)
    cosK = nc.declare_dram_parameter("cosK", [HALF, NQ], BF, isOutput=False)
    sinK = nc.declare_dram_parameter("sinK", [HALF, NQ], BF, isOutput=False)
    masks = nc.declare_dram_parameter("masks", [NSLOT, 2, KB, QB], BF, isOutput=False)
    out = nc.declare_dram_parameter("out", [NQ, D], F32, isOutput=True)

    ld = nc.sync       # streaming loads (q blocks, cos/sin, kt/vt, masks, wo)
    pf = nc.scalar     # prefetch loads (weight halves, first-step K/V)
    st = nc.gpsimd     # all stores + collectives

    with tile.TileContext(nc) as tc:
        from contextlib import ExitStack
        es = ExitStack()
        dram = es.enter_context(tc.tile_pool(name="dram", bufs=1, space="DRAM"))
        QT_d = dram.tile([D, NQ], BF, tag="QT_d")
        l_d = dram.tile([NSLOT, QB], F32, tag="l_d")
        kvK = [dram.tile([KTSZ], BF, tag=f"kvK{i}", name=f"kvK{i}")
               for i in range(4)]
        kvV = [dram.tile([KB * D], BF, tag=f"kvV{i}", name=f"kvV{i}")
               for i in range(4)]
        gK = [dram.tile([2, KTSZ], BF, tag=f"gK{i}", name=f"gK{i}")
              for i in range(4)]
        gV = [dram.tile([2, KB * D], BF, tag=f"gV{i}", name=f"gV{i}")
              for i in range(4)]

        def _ag(src_t, dst_t):
            nc.gpsimd.collective_compute(
                "AllGather",
                mybir.AluOpType.bypass,
                replica_groups=[[0, 1], [2, 3], [4, 5], [6, 7]],
                ins=[src_t[:].opt()],
                outs=[dst_t[:].opt()],
            )

        const = es.enter_context(tc.tile_pool(name="const", bufs=1))
        ones_t = const.tile([P, 1], BF, tag="ones")
        nc.vector.memset(ones_t, 1.0)

        # ---- persistent-ish pools ----
        wa_cm = tc.tile_pool(name="WA", bufs=1)                          # left
        wa = wa_cm.__enter__()
        wb_cm = tc.tile_pool(name="WB", bufs=1, side="right")
        wb = wb_cm.__enter__()

        # shared projection-phase working pools (left)
        qio_cm = tc.tile_pool(name="qio", bufs=2)
        qio = qio_cm.__enter__()
        csio_cm = tc.tile_pool(name="csio", bufs=2)
        csio = csio_cm.__enter__()

        def open_kotmp():
            ko_cm = tc.tile_pool(name="ko", bufs=3)
            tmp_cm = tc.tile_pool(name="tmp", bufs=2)
            return ko_cm, ko_cm.__enter__(), tmp_cm, tmp_cm.__enter__()

        def _w(w1, w2, di, cols):
            return (w1 if di < 8 else w2)[:, di % 8, cols]

        def _q(qa, qb, di):
            return (qa if di < 8 else qb)[:, di % 8]

        def load_w_pair(pool, src, eng1, eng2, tag1="w1", tag2="w2"):
            w1 = pool.tile([P, 8, D], BF, tag=tag1)
            w2 = pool.tile([P, 8, D], BF, tag=tag2)
            v = src.rearrange("(c p) f -> p c f", p=P)
            for c in range(8):
                eng1.dma_start(w1[:, c], v[:, c])
            for c in range(8):
                eng2.dma_start(w2[:, c], v[:, c + 8])
            return w1, w2

        # ================= K projection =================
        # Wk halves split across both queues for fast startup; Wv prefetch
        # (right slot) and block-0 cos/sin go to the store queue, idle now.
        ko_cm, kop, tmp_cm, tmpp = open_kotmp()

        w1, w2 = load_w_pair(wa, Wk, ld, pf)
        wv1 = wb.tile([P, 8, D], BF, tag="wv1")
        wv2 = wb.tile([P, 8, D], BF, tag="wv2")
        vv_ = Wv.rearrange("(c p) f -> p c f", p=P)

        def rope_block(qsrc_cols, cos_cols, sin_cols, w1, w2, emit,
                       q_eng=ld, cs_eng=ld, issue_after_q=None):
            qa = qio.tile([P, 8, QB], BF, tag="qa")
            qb = qio.tile([P, 8, QB], BF, tag="qb")
            vq = qsrc_cols.rearrange("(c p) f -> p c f", p=P)
            for c in range(8):
                q_eng.dma_start(qa[:, c], vq[:, c])
            for c in range(8):
                q_eng.dma_start(qb[:, c], vq[:, c + 8])
            if issue_after_q is not None:
                issue_after_q()
            cos_t = csio.tile([P, 8, QB], BF, tag="cos")
            sin_t = csio.tile([P, 8, QB], BF, tag="sin")
            _dma_in(cs_eng, cos_t, cos_cols, 8)
            _dma_in(cs_eng, sin_t, sin_cols, 8)
            for j in range(8):
                psA = pps.tile([P, QB], F32, tag="psA")
                psB = pps.tile([P, QB], F32, tag="psB")
                for di in range(DI):
                    nc.tensor.matmul(
                        psA, _w(w1, w2, di, slice(j * P, (j + 1) * P)),
                        _q(qa, qb, di),
                        start=(di == 0), stop=(di == DI - 1))
                for di in range(DI):
                    nc.tensor.matmul(
                        psB, _w(w1, w2, di, slice((j + 8) * P, (j + 9) * P)),
                        _q(qa, qb, di),
                        start=(di == 0), stop=(di == DI - 1))
                t1 = tmpp.tile([P, QB], F32, tag="t1")
                t2 = tmpp.tile([P, QB], F32, tag="t2")
                nc.vector.tensor_tensor(t1, psA, cos_t[:, j],
                                        mybir.AluOpType.mult)
                nc.vector.tensor_tensor(t2, psB, sin_t[:, j],
                                        mybir.AluOpType.mult)
                emit(j, t1, t2, True)
                nc.vector.tensor_tensor(t1, psA, sin_t[:, j],
                                        mybir.AluOpType.mult)
                nc.vector.tensor_tensor(t2, psB, cos_t[:, j],
                                        mybir.AluOpType.mult)
                emit(j + 8, t1, t2, False)

        with tc.tile_pool(name="pps", bufs=3, space="PSUM") as pps:
            for kb in range(4):
                def issue_wv():
                    # issue after block 1's loads so it doesn't steal startup
                    # bandwidth; transfers during K blocks 1-3, used in V.
                    # On the sync queue: the Act HWDGE queue posts completion
                    # semaphores so late that anything gating on its ring
                    # stalls for tens of us.
                    if kb == 1:
                        for c in range(8):
                            ld.dma_start(wv1[:, c], vv_[:, c])
                        for c in range(8):
                            ld.dma_start(wv2[:, c], vv_[:, c + 8])

                kv_view = kvK[kb][:].rearrange("(c p f) -> p c f", p=P, f=KB)

                def emit_k(ch, t1, t2, is_sub, kv_view=kv_view):
                    ko = kop.tile([P, QB], BF, tag="ko1" if is_sub else "ko2")
                    nc.vector.tensor_tensor(
                        ko, t1, t2,
                        mybir.AluOpType.subtract if is_sub
                        else mybir.AluOpType.add)
                    st.dma_start(kv_view[:, ch], ko)

                sl = slice(kb * KB, (kb + 1) * KB)
                rope_block(qT_keys[:, sl], cosK[:, sl], sinK[:, sl],
                           w1, w2, emit_k,
                           q_eng=(st if kb == 0 else ld),
                           cs_eng=(st if kb == 0 else ld),
                           issue_after_q=issue_wv)
                _ag(kvK[kb], gK[kb])

        # Wq prefetch into WA (waits on Wk's last reader; rides the store
        # queue behind the K AllGathers — transfers early in V, used in Q)
        wq1, wq2 = load_w_pair(wa, Wq, st, st)

        # ================= V projection =================
        tmp_cm.__exit__(None, None, None)
        ko_cm.__exit__(None, None, None)
        with (
            tc.tile_pool(name="vo", bufs=3) as vop,
            tc.tile_pool(name="vps", bufs=6, space="PSUM") as vps,
        ):
            # load q blocks one iteration ahead so the data-dependent vv
            # stores (same sync queue) never head-of-line block them
            def load_vq(kb):
                qa = qio.tile([P, 8, QB], BF, tag="qa", name=f"vqa{kb}")
                qb = qio.tile([P, 8, QB], BF, tag="qb", name=f"vqb{kb}")
                vq = qT_keys[:, kb * KB:(kb + 1) * KB].rearrange(
                    "(c p) f -> p c f", p=P)
                for c in range(8):
                    ld.dma_start(qa[:, c], vq[:, c])
                for c in range(8):
                    ld.dma_start(qb[:, c], vq[:, c + 8])
                return qa, qb

            nxt = load_vq(0)
            for kb in range(4):
                qa, qb = nxt
                if kb < 3:
                    nxt = load_vq(kb + 1)
                vv = kvV[kb][:].rearrange("(s d) -> s d", d=D)
                for ss in range(4):
                    vo = vop.tile([P, D], BF, tag="vo")
                    for dob in range(4):
                        ps = vps.tile([P, QB], F32, tag="vps")
                        for di in range(DI):
                            nc.tensor.matmul(
                                ps, _q(qa, qb, di)[:, ss * P:(ss + 1) * P],
                                _w(wv1, wv2, di,
                                   slice(dob * QB, (dob + 1) * QB)),
                                start=(di == 0), stop=(di == DI - 1))
                        nc.any.tensor_copy(vo[:, dob * QB:(dob + 1) * QB], ps)
                    ld.dma_start(vv[ss * P:(ss + 1) * P, :], vo)
                _ag(kvV[kb], gV[kb])

        # ================= Q projection =================
        # WB (right) closes; the prefetch pool takes its place holding slot
        # 3's Q (written directly by the epilogue) and the first attention
        # step's K/V tiles (DMA'd during Q-proj).
        wb_cm.__exit__(None, None, None)
        pre_cm = tc.tile_pool(name="prefetch", bufs=1, side="right")
        prep = pre_cm.__enter__()
        q3 = prep.tile([P, DI, QB], BF, tag="q3")
        kt0 = prep.tile([P, DI, KB], BF, tag="kt0")
        vt0 = prep.tile([P, 4, D], BF, tag="vt0")
        ag0, gi0 = AGIDX[0]

        ko_cm, kop, tmp_cm, tmpp = open_kotmp()
        with tc.tile_pool(name="pps", bufs=3, space="PSUM") as pps:
            for sb in SLOT_ORDER:
                if sb == 2:
                    # first attention step's K/V: issue after slot 3's own
                    # loads, transfers during the rest of Q-proj (sync queue)
                    _dma_in(ld, kt0,
                            gK[ag0][gi0].rearrange("(d s) -> d s", s=KB), DI)
                    _dma_in(ld, vt0,
                            gV[ag0][gi0].rearrange("(s d) -> s d", d=D), 4)
                sl = slice(sb * QB, (sb + 1) * QB)
                if sb == 3:
                    def emit_q(ch, t1, t2, is_sub):
                        nc.vector.tensor_tensor(
                            q3[:, ch], t1, t2,
                            mybir.AluOpType.subtract if is_sub
                            else mybir.AluOpType.add)
                else:
                    qv = QT_d[:, sl].rearrange("(c p) f -> p c f", p=P)

                    def emit_q(ch, t1, t2, is_sub, qv=qv):
                        ko = kop.tile([P, QB], BF,
                                      tag="ko1" if is_sub else "ko2")
                        nc.vector.tensor_tensor(
                            ko, t1, t2,
                            mybir.AluOpType.subtract if is_sub
                            else mybir.AluOpType.add)
                        st.dma_start(qv[:, ch], ko)

                rope_block(qT_own[:, sl], cosO[:, sl], sinO[:, sl],
                           wq1, wq2, emit_q)
        tmp_cm.__exit__(None, None, None)
        ko_cm.__exit__(None, None, None)
        csio_cm.__exit__(None, None, None)
        qio_cm.__exit__(None, None, None)
        wa_cm.__exit__(None, None, None)

        # ================= attention + output projection =================
        with (
            tc.tile_pool(name="qslot", bufs=1) as qslot,
            tc.tile_pool(name="kio", bufs=2) as kio,
            tc.tile_pool(name="vio", bufs=2) as vio,
            tc.tile_pool(name="pt", bufs=2) as ptpool,
            tc.tile_pool(name="mio", bufs=1) as mio,
            tc.tile_pool(name="ot", bufs=1) as otpool,
            tc.tile_pool(name="wo", bufs=2) as wopool,
            tc.tile_pool(name="fo", bufs=2) as fopool,
            tc.tile_pool(name="small", bufs=2) as small,
            tc.tile_pool(name="stps", bufs=2, space="PSUM") as stps,
            tc.tile_pool(name="pvps", bufs=2, space="PSUM") as pvps,
            tc.tile_pool(name="fps", bufs=2, space="PSUM") as fps,
            tc.tile_pool(name="lps", bufs=1, space="PSUM") as lps,
        ):
            for j in SLOT_ORDER:
                t = TSTEPS[j]
                if j == 3:
                    q_t = q3
                else:
                    q_t = qslot.tile([P, DI, QB], BF, tag="qs")
                    _dma_in(ld, q_t, QT_d[:, j * QB:(j + 1) * QB], DI)
                m_t = mio.tile([P, 2, 4, QB], BF, tag="mask")
                for sx in range(2):
                    _dma_in(ld, m_t[:, sx], masks[j, sx], 4)
                # per-chunk accumulator tiles: O-proj matmuls depend only on
                # the chunks they read, not on every PV add of the slot
                ot = [otpool.tile([P, QB], BF, tag=f"ot{do}",
                                  name=f"ot{do}_{j}")
                      for do in range(NCH)]
                l_ps = lps.tile([1, QB], F32, tag="lps")
                wo_pre = []
                for s in range(t):
                    ag, idx = AGIDX[s]
                    if s == t - 1:
                        # prefetch the first two Wo column groups behind the
                        # final step's K/V loads (nothing else needs the
                        # queue until the next slot's q) so O-proj never
                        # waits on them
                        for dob in range(2):
                            wo_t = wopool.tile([P, DI, QB], BF, tag="wo",
                                               name=f"wo{dob}_{j}")
                            _dma_in(ld, wo_t,
                                    Wo[:, dob * QB:(dob + 1) * QB], DI)
                            wo_pre.append(wo_t)
                    if j == 3 and s == 0:
                        kt, vt = kt0, vt0
                    else:
                        kt = kio.tile([P, DI, KB], BF, tag="kt")
                        _dma_in(ld, kt,
                                gK[ag][idx].rearrange("(d s) -> d s", s=KB),
                                DI)
                        vt = vio.tile([P, 4, D], BF, tag="vt")
                        _dma_in(ld, vt,
                                gV[ag][idx].rearrange("(s d) -> s d", d=D), 4)
                    pt = ptpool.tile([P, 4, QB], BF, tag="pt")
                    final = (s == t - 1)
                    masked = s >= t - 2
                    for kc in range(4):
                        off = kc * P if final else 0
                        stile = stps.tile([P, QB], F32, tag="st")
                        for di in range(DI):
                            nc.tensor.matmul(
                                stile[:, off:],
                                kt[:, di, kc * P:(kc + 1) * P],
                                q_t[:, di, off:],
                                start=(di == 0), stop=(di == DI - 1))
                        if masked:
                            nc.vector.tensor_tensor(
                                stile[:, off:], stile[:, off:],
                                m_t[:, s - (t - 2), kc, off:],
                                mybir.AluOpType.add)
                        nc.scalar.activation(
                            pt[:, kc, off:], stile[:, off:],
                            mybir.ActivationFunctionType.Exp)
                        nc.tensor.matmul(
                            l_ps[:, off:], ones_t, pt[:, kc, off:],
                            start=(s == 0 and kc == 0),
                            stop=(final and kc == 3),
                            skip_group_check=True)
                    for do in range(NCH):
                        pv = pvps.tile([P, QB], F32, tag="pv")
                        for kc in range(4):
                            off = kc * P if final else 0
                            nc.tensor.matmul(
                                pv[:, off:],
                                vt[:, kc, do * P:(do + 1) * P],
                                pt[:, kc, off:],
                                start=(kc == 0), stop=(kc == 3),
                                skip_group_check=final)
                        if s == 0:
                            nc.vector.tensor_copy(ot[do], pv)
                        else:
                            nc.vector.tensor_tensor(
                                ot[do], ot[do], pv,
                                mybir.AluOpType.add)
                # O projection: bounce l through DRAM to transpose
                # [1, 512] -> per-partition columns [P, 4] (store queue).
                l_sb = small.tile([1, QB], F32, tag="lsb")
                nc.any.tensor_copy(l_sb, l_ps)
                st.dma_start(l_d[j:j + 1, :], l_sb)
                lcols = small.tile([P, 4], F32, tag="lcols")
                st.dma_start(lcols, l_d[j].rearrange("(qs p) -> p qs", p=P))
                inv_l = small.tile([P, 4], F32, tag="invl")
                nc.vector.reciprocal(inv_l, lcols)
                for dob in range(4):
                    if dob < 2:
                        wo_t = wo_pre[dob]
                    else:
                        wo_t = wopool.tile([P, DI, QB], BF, tag="wo")
                        _dma_in(ld, wo_t, Wo[:, dob * QB:(dob + 1) * QB], DI)
                    for qs in range(4):
                        f_ps = fps.tile([P, QB], F32, tag="fps")
                        for di in range(DI):
                            nc.tensor.matmul(
                                f_ps, ot[di][:, qs * P:(qs + 1) * P],
                                wo_t[:, di, :],
                                start=(di == 0), stop=(di == DI - 1))
                        fo = fopool.tile([P, QB], F32, tag="fo")
                        nc.vector.tensor_scalar_mul(
                            fo, f_ps, inv_l[:, qs:qs + 1])
                        st.dma_start(
                            out[j * QB + qs * P: j * QB + (qs + 1) * P,
                                dob * QB:(dob + 1) * QB], fo)
        pre_cm.__exit__(None, None, None)
        es.close()
    return nc


_NC_CACHE = None


def _get_nc():
    global _NC_CACHE
    if _NC_CACHE is None:
        _NC_CACHE = _build()
    return _NC_CACHE


def _host_prep(q, W_q, W_k, W_v, W_o):
    perm = np.concatenate([np.arange(0, D, 2), np.arange(1, D, 2)])
    scale = 1.0 / math.sqrt(D)
    Wq_p = np.ascontiguousarray((W_q * scale)[:, perm]).astype(bf16)
    Wk_p = np.ascontiguousarray(W_k[:, perm]).astype(bf16)
    Wv_p = W_v.astype(bf16)
    Wo_p = W_o.astype(bf16)
    inv_freq = 1.0 / (ROPE_BASE ** (np.arange(0, D, 2, dtype=np.float64) / D))
    ang = np.arange(S, dtype=np.float64)[:, None] * inv_freq[None, :]
    cosT = np.ascontiguousarray(np.cos(ang).T).astype(bf16)   # (HALF, S)
    sinT = np.ascontiguousarray(np.sin(ang).T).astype(bf16)
    return Wq_p, Wk_p, Wv_p, Wo_p, cosT, sinT


def _make_masks(blocks):
    m = np.zeros((NSLOT, 2, KB, QB), dtype=np.float32)
    k_idx = np.arange(KB)[:, None]
    q_idx = np.arange(QB)[None, :]
    tri = np.where(k_idx <= q_idx, 0.0, NEG).astype(np.float32)
    for j, blk in enumerate(blocks):
        t = TSTEPS[j]
        limit = blk + 1
        for sidx, s in enumerate([t - 2, t - 1]):
            if s == limit - 1:
                m[j, sidx] = tri
            elif s >= limit:
                m[j, sidx] = NEG
    return m.astype(bf16)


def run(inputs, trace=False):
    q = np.asarray(inputs["q"], dtype=np.float32)
    W_q = np.asarray(inputs["W_q"], dtype=np.float32)
    W_k = np.asarray(inputs["W_k"], dtype=np.float32)
    W_v = np.asarray(inputs["W_v"], dtype=np.float32)
    W_o = np.asarray(inputs["W_o"], dtype=np.float32)

    Wq_p, Wk_p, Wv_p, Wo_p, cosT, sinT = _host_prep(q, W_q, W_k, W_v, W_o)

    in_maps = []
    core_blocks = []
    for c in range(8):
        b = c // 2
        blocks = BLOCKS_EVEN if c % 2 == 0 else BLOCKS_ODD
        keys = KEYS_EVEN if c % 2 == 0 else KEYS_ODD
        core_blocks.append((b, blocks))
        qTb = np.ascontiguousarray(q[b].T).astype(bf16)       # (D, S)
        own_cols = np.concatenate(
            [np.arange(blk * QB, (blk + 1) * QB) for blk in blocks])
        key_cols = np.concatenate(
            [np.arange(blk * QB, (blk + 1) * QB) for blk in keys])
        in_maps.append({
            "qT_own": np.ascontiguousarray(qTb[:, own_cols]),
            "qT_keys": np.ascontiguousarray(qTb[:, key_cols]),
            "Wq": Wq_p, "Wk": Wk_p, "Wv": Wv_p, "Wo": Wo_p,
            "cosO": np.ascontiguousarray(cosT[:, own_cols]),
            "sinO": np.ascontiguousarray(sinT[:, own_cols]),
            "cosK": np.ascontiguousarray(cosT[:, key_cols]),
            "sinK": np.ascontiguousarray(sinT[:, key_cols]),
            "masks": _make_masks(blocks),
        })

    nc = _get_nc()
    res = run_bass_kernel_spmd(nc, in_maps, core_ids=list(range(8)),
                               trace=trace)

    out = np.zeros((B, S, D), dtype=np.float32)
    for c, (b, blocks) in enumerate(core_blocks):
        o = res.results[c]["out"]
        for j, blk in enumerate(blocks):
            out[b, blk * QB:(blk + 1) * QB] = o[j * QB:(j + 1) * QB]
    return out, res


def kernel(**inputs):
    return run(inputs, trace=False)[0]


# revision 34
# speedup vs baseline: 1.1648x; 1.0095x over previous
"""Trainium2 Bass kernel for single-head causal attention with RoPE.

Problem: B=4, S=4096, D=2048, H=1.
  out = softmax(causal(rope(q@Wq) @ rope(q@Wk)^T / sqrt(D))) @ (q@Wv) @ Wo

Sharding: 8 cores = 4 batches x 2 groups. Each core owns 4 of the batch's 8
512-row blocks ({7,4,3,0} even cores / {6,5,2,1} odd) — a causal-balanced
split: both cores see 18 real key-steps, padded to a uniform per-slot
schedule TSTEPS=[8,6,4,2] so all cores run one NEFF. K/V projections are
computed only for OWN blocks; the pair exchanges K/V per block-pair through
four 2-rank AllGathers that overlap later projection phases.

All matmuls bf16 (PSUM accumulates fp32). RoPE is reduced to half-rotation by
permuting Wq/Wk columns on the host; the score scale 1/sqrt(D) is folded into
Wq. Attention is transpose-free: scores are computed as S^T[k,q] per block,
softmax runs without max-subtraction, the denominator comes from per-qs
pt^T@ones matmuls accumulated directly in [P,4] orientation, and P^T feeds
the PV matmul directly.

Perf structure (vs the first working version):
- Two rotating weight slots (WA left / WB right): Wv prefetches during
  K-proj, Wq during V-proj, so phase transitions never stall on weight DMA.
- DMA queues split by role: sync=streaming loads, scalar=weight prefetch +
  first-attention-step prefetch, gpsimd=all stores + collectives. This avoids
  head-of-line blocking of latency-critical loads behind data-dependent
  stores.
- Slot 3's Q stays in SBUF (no DRAM round trip); first attention step's K/V
  tiles prefetch during Q-proj.
- The FINAL key-step of every slot is triangle-trimmed: per kc chunk only
  queries >= kc*128 are computed in QK/exp/l/PV (on even cores that step is
  the fully-masked pad step; trimming is a superset of its needs).
- ot accumulates in bf16 (no separate cast tile before the O projection).
"""

import json
import math

import ml_dtypes
import numpy as np

import concourse.bass as bass
import concourse.mybir as mybir
import concourse.tile as tile
from concourse.bass_utils import run_bass_kernel_spmd


def _split_multi_waits(bir_json_bytes):
    """Rewrite BIR so no instruction carries more than one semaphore wait.

    The walrus build in this environment rejects instructions with >1 sync
    wait. Extra waits are hoisted onto injected same-engine EventSemaphore
    instructions placed immediately before the instruction (engine program
    order makes them gate it)."""
    d = json.loads(bir_json_bytes)
    for fn in d["functions"]:
        for blk in fn["blocks"]:
            out = []
            for inst in blk["instructions"]:
                si = inst.get("sync_info") or {}
                ow = si.get("on_wait") or []
                if len(ow) > 1:
                    for i, w in enumerate(ow[:-1]):
                        out.append({
                            "debug": inst.get("debug"),
                            "engine": inst["engine"],
                            "ins": [],
                            "outs": [],
                            "name": f"{inst['name']}_sw{i}",
                            "opcode": "EventSemaphore",
                            "sync_info": {"on_update": [], "on_wait": [w]},
                        })
                    si["on_wait"] = [ow[-1]]
                out.append(inst)
            blk["instructions"] = out
    return json.dumps(d).encode()


def _install_split_waits():
    import concourse.bass_utils as bu
    if getattr(bu, "_split_waits_installed", False):
        return
    orig = bu.compile_bir_kernel

    def patched(bir_json, tmpdir, neff_name="file.neff"):
        return orig(_split_multi_waits(bir_json), tmpdir, neff_name)

    bu.compile_bir_kernel = patched
    bu._split_waits_installed = True
    import concourse.bass2jax as b2j
    if getattr(b2j, "compile_bir_kernel", None) is orig:
        b2j.compile_bir_kernel = patched


_install_split_waits()

BF = mybir.dt.bfloat16
F32 = mybir.dt.float32
bf16 = ml_dtypes.bfloat16

B, S, D = 4, 4096, 2048
HALF = D // 2
P = 128
QB = 512           # query block (one slot)
KB = 512           # key step
NSLOT = 4          # query blocks per core
NQ = NSLOT * QB    # 2048 own queries per core
DI = D // P        # 16 contraction chunks
NCH = D // P       # 16 output chunks
TSTEPS = [8, 6, 4, 2]   # padded key steps per slot (uniform across cores)
SLOT_ORDER = [3, 2, 1, 0]   # processing order: ascending step counts
BLOCKS_EVEN = [7, 4, 3, 0]
BLOCKS_ODD = [6, 5, 2, 1]
KEYS_EVEN = [0, 3, 4, 7]   # same blocks, ascending (AllGather pairing order)
KEYS_ODD = [1, 2, 5, 6]
# key block s of the batch lives at gathered[AGIDX[s][0]][AGIDX[s][1]]
AGIDX = [(0, 0), (0, 1), (1, 1), (1, 0), (2, 0), (2, 1), (3, 1), (3, 0)]
KTSZ = D * KB              # elements of one K^T block [D, 512]
ROPE_BASE = 10000.0
NEG = -1.0e30


def _dma_in(eng, dst, src_ap, n):
    """Per-chunk DMA load of a [P, n, F] tile from a "(c p) f" DRAM view."""
    v = src_ap.rearrange("(c p) f -> p c f", p=P)
    for c in range(n):
        eng.dma_start(dst[:, c], v[:, c])


def _build():
    nc = bass.Bass(num_devices=8)

    qT_own = nc.declare_dram_parameter("qT_own", [D, NQ], BF, isOutput=False)
    qT_keys = nc.declare_dram_parameter("qT_keys", [D, NQ], BF, isOutput=False)
    Wq = nc.declare_dram_parameter("Wq", [D, D], BF, isOutput=False)
    Wk = nc.declare_dram_parameter("Wk", [D, D], BF, isOutput=False)
    Wv = nc.declare_dram_parameter("Wv", [D, D], BF, isOutput=False)
    Wo = nc.declare_dram_parameter("Wo", [D, D], BF, isOutput=False)
    cosO = nc.declare_dram_parameter("cosO", [HALF, NQ], BF, isOutput=False)
    sinO = nc.declare_dram_parameter("sinO", [HALF, NQ], BF, isOutput=False)
    cosK = nc.declare_dram_parameter("cosK", [HALF, NQ], BF, isOutput=False)
    sinK = nc.declare_dram_parameter("sinK", [HALF, NQ], BF, isOutput=False)
    masks = nc.declare_dram_parameter("masks", [NSLOT, 2, KB, QB], BF, isOutput=False)
    out = nc.declare_dram_parameter("out", [NQ, D], F32, isOutput=True)

    ld = nc.sync       # streaming loads (q blocks, cos/sin, kt/vt, masks, wo)
    pf = nc.scalar     # prefetch loads (weight halves, first-step K/V)
    st = nc.gpsimd     # all stores + collectives

    with tile.TileContext(nc) as tc:
        from contextlib import ExitStack
        es = ExitStack()
        dram = es.enter_context(tc.tile_pool(name="dram", bufs=1, space="DRAM"))
        QT_d = dram.tile([D, NQ], BF, tag="QT_d")
        l_d = dram.tile([NSLOT, QB], F32, tag="l_d")
        kvK = [dram.tile([KTSZ], BF, tag=f"kvK{i}", name=f"kvK{i}")
               for i in range(4)]
        kvV = [dram.tile([KB * D], BF, tag=f"kvV{i}", name=f"kvV{i}")
               for i in range(4)]
        gK = [dram.tile([2, KTSZ], BF, tag=f"gK{i}", name=f"gK{i}")
              for i in range(4)]
        gV = [dram.tile([2, KB * D], BF, tag=f"gV{i}", name=f"gV{i}")
              for i in range(4)]

        def _ag(src_t, dst_t):
            nc.gpsimd.collective_compute(
                "AllGather",
                mybir.AluOpType.bypass,
                replica_groups=[[0, 1], [2, 3], [4, 5], [6, 7]],
                ins=[src_t[:].opt()],
                outs=[dst_t[:].opt()],
            )

        const = es.enter_context(tc.tile_pool(name="const", bufs=1))
        ones_t = const.tile([P, 1], BF, tag="ones")
        nc.vector.memset(ones_t, 1.0)

        # ---- persistent-ish pools ----
        wa_cm = tc.tile_pool(name="WA", bufs=1)                          # left
        wa = wa_cm.__enter__()
        wb_cm = tc.tile_pool(name="WB", bufs=1, side="right")
        wb = wb_cm.__enter__()

        # shared projection-phase working pools (left)
        qio_cm = tc.tile_pool(name="qio", bufs=2)
        qio = qio_cm.__enter__()
        csio_cm = tc.tile_pool(name="csio", bufs=2)
        csio = csio_cm.__enter__()

        def open_kotmp():
            ko_cm = tc.tile_pool(name="ko", bufs=3)
            tmp_cm = tc.tile_pool(name="tmp", bufs=2)
            return ko_cm, ko_cm.__enter__(), tmp_cm, tmp_cm.__enter__()

        def _w(w1, w2, di, cols):
            return (w1 if di < 8 else w2)[:, di % 8, cols]

        def _q(qa, qb, di):
            return (qa if di < 8 else qb)[:, di % 8]

        def load_w_pair(pool, src, eng1, eng2, tag1="w1", tag2="w2"):
            w1 = pool.tile([P, 8, D], BF, tag=tag1)
            w2 = pool.tile([P, 8, D], BF, tag=tag2)
            v = src.rearrange("(c p) f -> p c f", p=P)
            for c in range(8):
                eng1.dma_start(w1[:, c], v[:, c])
            for c in range(8):
                eng2.dma_start(w2[:, c], v[:, c + 8])
            return w1, w2

        # ================= K projection =================
        # Wk halves split across both queues for fast startup; Wv prefetch
        # (right slot) and block-0 cos/sin go to the store queue, idle now.
        ko_cm, kop, tmp_cm, tmpp = open_kotmp()

        w1, w2 = load_w_pair(wa, Wk, ld, pf)
        wv1 = wb.tile([P, 8, D], BF, tag="wv1")
        wv2 = wb.tile([P, 8, D], BF, tag="wv2")
        vv_ = Wv.rearrange("(c p) f -> p c f", p=P)

        def rope_block(qsrc_cols, cos_cols, sin_cols, w1, w2, emit,
                       q_eng=ld, cs_eng=ld, issue_after_q=None):
            qa = qio.tile([P, 8, QB], BF, tag="qa")
            qb = qio.tile([P, 8, QB], BF, tag="qb")
            vq = qsrc_cols.rearrange("(c p) f -> p c f", p=P)
            for c in range(8):
                q_eng.dma_start(qa[:, c], vq[:, c])
            for c in range(8):
                q_eng.dma_start(qb[:, c], vq[:, c + 8])
            if issue_after_q is not None:
                issue_after_q()
            cos_t = csio.tile([P, 8, QB], BF, tag="cos")
            sin_t = csio.tile([P, 8, QB], BF, tag="sin")
            _dma_in(cs_eng, cos_t, cos_cols, 8)
            _dma_in(cs_eng, sin_t, sin_cols, 8)
            for j in range(8):
                psA = pps.tile([P, QB], F32, tag="psA")
                psB = pps.tile([P, QB], F32, tag="psB")
                for di in range(DI):
                    nc.tensor.matmul(
                        psA, _w(w1, w2, di, slice(j * P, (j + 1) * P)),
                        _q(qa, qb, di),
                        start=(di == 0), stop=(di == DI - 1))
                for di in range(DI):
                    nc.tensor.matmul(
                        psB, _w(w1, w2, di, slice((j + 8) * P, (j + 9) * P)),
                        _q(qa, qb, di),
                        start=(di == 0), stop=(di == DI - 1))
                t1 = tmpp.tile([P, QB], F32, tag="t1")
                t2 = tmpp.tile([P, QB], F32, tag="t2")
                nc.vector.tensor_tensor(t1, psA, cos_t[:, j],
                                        mybir.AluOpType.mult)
                nc.vector.tensor_tensor(t2, psB, sin_t[:, j],
                                        mybir.AluOpType.mult)
                emit(j, t1, t2, True)
                nc.vector.tensor_tensor(t1, psA, sin_t[:, j],
                                        mybir.AluOpType.mult)
                nc.vector.tensor_tensor(t2, psB, cos_t[:, j],
                                        mybir.AluOpType.mult)
                emit(j + 8, t1, t2, False)

        with tc.tile_pool(name="pps", bufs=3, space="PSUM") as pps:
            for kb in range(4):
                def issue_wv():
                    # issue after block 1's loads so it doesn't steal startup
                    # bandwidth; transfers during K blocks 1-3, used in V.
                    # On the sync queue: the Act HWDGE queue posts completion
                    # semaphores so late that anything gating on its ring
                    # stalls for tens of us.
                    if kb == 1:
                        for c in range(8):
                            ld.dma_start(wv1[:, c], vv_[:, c])
                        for c in range(8):
                            ld.dma_start(wv2[:, c], vv_[:, c + 8])

                kv_view = kvK[kb][:].rearrange("(c p f) -> p c f", p=P, f=KB)

                def emit_k(ch, t1, t2, is_sub, kv_view=kv_view):
                    ko = kop.tile([P, QB], BF, tag="ko1" if is_sub else "ko2")
                    nc.vector.tensor_tensor(
                        ko, t1, t2,
                        mybir.AluOpType.subtract if is_sub
                        else mybir.AluOpType.add)
                    st.dma_start(kv_view[:, ch], ko)

                sl = slice(kb * KB, (kb + 1) * KB)
                rope_block(qT_keys[:, sl], cosK[:, sl], sinK[:, sl],
                           w1, w2, emit_k,
                           q_eng=(st if kb == 0 else ld),
                           cs_eng=(st if kb == 0 else ld),
                           issue_after_q=issue_wv)
                _ag(kvK[kb], gK[kb])

        # Wq prefetch into WA (waits on Wk's last reader; rides the store
        # queue behind the K AllGathers — transfers early in V, used in Q)
        wq1, wq2 = load_w_pair(wa, Wq, st, st)

        # ================= V projection =================
        tmp_cm.__exit__(None, None, None)
        ko_cm.__exit__(None, None, None)
        with (
            tc.tile_pool(name="vo", bufs=3) as vop,
            tc.tile_pool(name="vps", bufs=6, space="PSUM") as vps,
        ):
            # load q blocks one iteration ahead so the data-dependent vv
            # stores (same sync queue) never head-of-line block them
            def load_vq(kb):
                qa = qio.tile([P, 8, QB], BF, tag="qa", name=f"vqa{kb}")
                qb = qio.tile([P, 8, QB], BF, tag="qb", name=f"vqb{kb}")
                vq = qT_keys[:, kb * KB:(kb + 1) * KB].rearrange(
                    "(c p) f -> p c f", p=P)
                for c in range(8):
                    ld.dma_start(qa[:, c], vq[:, c])
                for c in range(8):
                    ld.dma_start(qb[:, c], vq[:, c + 8])
                return qa, qb

            nxt = load_vq(0)
            for kb in range(4):
                qa, qb = nxt
                if kb < 3:
                    nxt = load_vq(kb + 1)
                vv = kvV[kb][:].rearrange("(s d) -> s d", d=D)
                for ss in range(4):
                    vo = vop.tile([P, D], BF, tag="vo")
                    for dob in range(4):
                        ps = vps.tile([P, QB], F32, tag="vps")
                        for di in range(DI):
                            nc.tensor.matmul(
                                ps, _q(qa, qb, di)[:, ss * P:(ss + 1) * P],
                                _w(wv1, wv2, di,
                                   slice(dob * QB, (dob + 1) * QB)),
                                start=(di == 0), stop=(di == DI - 1))
                        nc.any.tensor_copy(vo[:, dob * QB:(dob + 1) * QB], ps)
                    ld.dma_start(vv[ss * P:(ss + 1) * P, :], vo)
                _ag(kvV[kb], gV[kb])

        # ================= Q projection =================
        # WB (right) closes; the prefetch pool takes its place holding slot
        # 3's Q (written directly by the epilogue) and the first attention
        # step's K/V tiles (DMA'd during Q-proj).
        wb_cm.__exit__(None, None, None)
        pre_cm = tc.tile_pool(name="prefetch", bufs=1, side="right")
        prep = pre_cm.__enter__()
        q3 = prep.tile([P, DI, QB], BF, tag="q3")
        kt0 = prep.tile([P, DI, KB], BF, tag="kt0")
        vt0 = prep.tile([P, 4, D], BF, tag="vt0")
        ag0, gi0 = AGIDX[0]

        ko_cm, kop, tmp_cm, tmpp = open_kotmp()
        with tc.tile_pool(name="pps", bufs=3, space="PSUM") as pps:
            for sb in SLOT_ORDER:
                if sb == 2:
                    # first attention step's K/V: issue after slot 3's own
                    # loads, transfers during the rest of Q-proj (sync queue)
                    _dma_in(ld, kt0,
                            gK[ag0][gi0].rearrange("(d s) -> d s", s=KB), DI)
                    _dma_in(ld, vt0,
                            gV[ag0][gi0].rearrange("(s d) -> s d", d=D), 4)
                sl = slice(sb * QB, (sb + 1) * QB)
                if sb == 3:
                    def emit_q(ch, t1, t2, is_sub):
                        nc.vector.tensor_tensor(
                            q3[:, ch], t1, t2,
                            mybir.AluOpType.subtract if is_sub
                            else mybir.AluOpType.add)
                else:
                    qv = QT_d[:, sl].rearrange("(c p) f -> p c f", p=P)

                    def emit_q(ch, t1, t2, is_sub, qv=qv):
                        ko = kop.tile([P, QB], BF,
                                      tag="ko1" if is_sub else "ko2")
                        nc.vector.tensor_tensor(
                            ko, t1, t2,
                            mybir.AluOpType.subtract if is_sub
                            else mybir.AluOpType.add)
                        st.dma_start(qv[:, ch], ko)

                rope_block(qT_own[:, sl], cosO[:, sl], sinO[:, sl],
                           wq1, wq2, emit_q)
        tmp_cm.__exit__(None, None, None)
        ko_cm.__exit__(None, None, None)
        csio_cm.__exit__(None, None, None)
        qio_cm.__exit__(None, None, None)
        wa_cm.__exit__(None, None, None)

        # ================= attention + output projection =================
        with (
            tc.tile_pool(name="qslot", bufs=1) as qslot,
            tc.tile_pool(name="kio", bufs=2) as kio,
            tc.tile_pool(name="vio", bufs=2) as vio,
            tc.tile_pool(name="pt", bufs=2) as ptpool,
            tc.tile_pool(name="mio", bufs=1) as mio,
            tc.tile_pool(name="ot", bufs=1) as otpool,
            tc.tile_pool(name="wo", bufs=2) as wopool,
            tc.tile_pool(name="fo", bufs=2) as fopool,
            tc.tile_pool(name="small", bufs=2) as small,
            tc.tile_pool(name="stps", bufs=2, space="PSUM") as stps,
            tc.tile_pool(name="pvps", bufs=2, space="PSUM") as pvps,
            tc.tile_pool(name="fps", bufs=2, space="PSUM") as fps,
            tc.tile_pool(name="lps", bufs=1, space="PSUM") as lps,
        ):
            for j in SLOT_ORDER:
                t = TSTEPS[j]
                if j == 3:
                    q_t = q3
                else:
                    q_t = qslot.tile([P, DI, QB], BF, tag="qs")
                    _dma_in(ld, q_t, QT_d[:, j * QB:(j + 1) * QB], DI)
                m_t = mio.tile([P, 2, 4, QB], BF, tag="mask")
                for sx in range(2):
                    _dma_in(ld, m_t[:, sx], masks[j, sx], 4)
                # per-chunk accumulator tiles: O-proj matmuls depend only on
                # the chunks they read, not on every PV add of the slot
                ot = [otpool.tile([P, QB], BF, tag=f"ot{do}",
                                  name=f"ot{do}_{j}")
                      for do in range(NCH)]
                l_ps = lps.tile([1, QB], F32, tag="lps")
                wo_pre = []
                for s in range(t):
                    ag, idx = AGIDX[s]
                    if j == 3 and s == 0:
                        kt, vt = kt0, vt0
                    else:
                        kt = kio.tile([P, DI, KB], BF, tag="kt")
                        _dma_in(ld, kt,
                                gK[ag][idx].rearrange("(d s) -> d s", s=KB),
                                DI)
                        vt = vio.tile([P, 4, D], BF, tag="vt")
                        _dma_in(ld, vt,
                                gV[ag][idx].rearrange("(s d) -> s d", d=D), 4)
                    if s == t - 1:
                        # prefetch the first two Wo column groups behind the
                        # final step's K/V loads (nothing else needs the
                        # queue until the next slot's q) so O-proj never
                        # waits on them
                        for dob in range(2):
                            wo_t = wopool.tile([P, DI, QB], BF, tag="wo",
                                               name=f"wo{dob}_{j}")
                            _dma_in(ld, wo_t,
                                    Wo[:, dob * QB:(dob + 1) * QB], DI)
                            wo_pre.append(wo_t)
                    pt = ptpool.tile([P, 4, QB], BF, tag="pt")
                    final = (s == t - 1)
                    masked = s >= t - 2
                    for kc in range(4):
                        off = kc * P if final else 0
                        stile = stps.tile([P, QB], F32, tag="st")
                        for di in range(DI):
                            nc.tensor.matmul(
                                stile[:, off:],
                                kt[:, di, kc * P:(kc + 1) * P],
                                q_t[:, di, off:],
                                start=(di == 0), stop=(di == DI - 1))
                        if masked:
                            nc.vector.tensor_tensor(
                                stile[:, off:], stile[:, off:],
                                m_t[:, s - (t - 2), kc, off:],
                                mybir.AluOpType.add)
                        nc.scalar.activation(
                            pt[:, kc, off:], stile[:, off:],
                            mybir.ActivationFunctionType.Exp)
                        nc.tensor.matmul(
                            l_ps[:, off:], ones_t, pt[:, kc, off:],
                            start=(s == 0 and kc == 0),
                            stop=(final and kc == 3),
                            skip_group_check=True)
                    for do in range(NCH):
                        pv = pvps.tile([P, QB], F32, tag="pv")
                        for kc in range(4):
                            off = kc * P if final else 0
                            nc.tensor.matmul(
                                pv[:, off:],
                                vt[:, kc, do * P:(do + 1) * P],
                                pt[:, kc, off:],
                                start=(kc == 0), stop=(kc == 3),
                                skip_group_check=final)
                        if s == 0:
                            nc.vector.tensor_copy(ot[do], pv)
                        else:
                            nc.vector.tensor_tensor(
                                ot[do], ot[do], pv,
                                mybir.AluOpType.add)
                # O projection: bounce l through DRAM to transpose
                # [1, 512] -> per-partition columns [P, 4] (store queue).
                l_sb = small.tile([1, QB], F32, tag="lsb")
                nc.any.tensor_copy(l_sb, l_ps)
                st.dma_start(l_d[j:j + 1, :], l_sb)
                lcols = small.tile([P, 4], F32, tag="lcols")
                st.dma_start(lcols, l_d[j].rearrange("(qs p) -> p qs", p=P))
                inv_l = small.tile([P, 4], F32, tag="invl")
                nc.vector.reciprocal(inv_l, lcols)
                for dob in range(4):
                    if dob < 2:
                        wo_t = wo_pre[dob]
                    else:
                        wo_t = wopool.tile([P, DI, QB], BF, tag="wo")
                        _dma_in(ld, wo_t, Wo[:, dob * QB:(dob + 1) * QB], DI)
                    for qs in range(4):
                        f_ps = fps.tile([P, QB], F32, tag="fps")
                        for di in range(DI):
                            nc.tensor.matmul(
                                f_ps, ot[di][:, qs * P:(qs + 1) * P],
                                wo_t[:, di, :],
                                start=(di == 0), stop=(di == DI - 1))
                        fo = fopool.tile([P, QB], F32, tag="fo")
                        nc.vector.tensor_scalar_mul(
                            fo, f_ps, inv_l[:, qs:qs + 1])
                        st.dma_start(
                            out[j * QB + qs * P: j * QB + (qs + 1) * P,
                                dob * QB:(dob + 1) * QB], fo)
        pre_cm.__exit__(None, None, None)
        es.close()
    return nc


_NC_CACHE = None


def _get_nc():
    global _NC_CACHE
    if _NC_CACHE is None:
        _NC_CACHE = _build()
    return _NC_CACHE


def _host_prep(q, W_q, W_k, W_v, W_o):
    perm = np.concatenate([np.arange(0, D, 2), np.arange(1, D, 2)])
    scale = 1.0 / math.sqrt(D)
    Wq_p = np.ascontiguousarray((W_q * scale)[:, perm]).astype(bf16)
    Wk_p = np.ascontiguousarray(W_k[:, perm]).astype(bf16)
    Wv_p = W_v.astype(bf16)
    Wo_p = W_o.astype(bf16)
    inv_freq = 1.0 / (ROPE_BASE ** (np.arange(0, D, 2, dtype=np.float64) / D))
    ang = np.arange(S, dtype=np.float64)[:, None] * inv_freq[None, :]
    cosT = np.ascontiguousarray(np.cos(ang).T).astype(bf16)   # (HALF, S)
    sinT = np.ascontiguousarray(np.sin(ang).T).astype(bf16)
    return Wq_p, Wk_p, Wv_p, Wo_p, cosT, sinT


def _make_masks(blocks):
    m = np.zeros((NSLOT, 2, KB, QB), dtype=np.float32)
    k_idx = np.arange(KB)[:, None]
    q_idx = np.arange(QB)[None, :]
    tri = np.where(k_idx <= q_idx, 0.0, NEG).astype(np.float32)
    for j, blk in enumerate(blocks):
        t = TSTEPS[j]
        limit = blk + 1
        for sidx, s in enumerate([t - 2, t - 1]):
            if s == limit - 1:
                m[j, sidx] = tri
            elif s >= limit:
                m[j, sidx] = NEG
    return m.astype(bf16)


def run(inputs, trace=False):
    q = np.asarray(inputs["q"], dtype=np.float32)
    W_q = np.asarray(inputs["W_q"], dtype=np.float32)
    W_k = np.asarray(inputs["W_k"], dtype=np.float32)
    W_v = np.asarray(inputs["W_v"], dtype=np.float32)
    W_o = np.asarray(inputs["W_o"], dtype=np.float32)

    Wq_p, Wk_p, Wv_p, Wo_p, cosT, sinT = _host_prep(q, W_q, W_k, W_v, W_o)

    in_maps = []
    core_blocks = []
    for c in range(8):
        b = c // 2
        blocks = BLOCKS_EVEN if c % 2 == 0 else BLOCKS_ODD
        keys = KEYS_EVEN if c % 2 == 0 else KEYS_ODD
        core_blocks.append((b, blocks))
        qTb = np.ascontiguousarray(q[b].T).astype(bf16)       # (D, S)
        own_cols = np.concatenate(
            [np.arange(blk * QB, (blk + 1) * QB) for blk in blocks])
        key_cols = np.concatenate(
            [np.arange(blk * QB, (blk + 1) * QB) for blk in keys])
        in_maps.append({
            "qT_own": np.ascontiguousarray(qTb[:, own_cols]),
            "qT_keys": np.ascontiguousarray(qTb[:, key_cols]),
            "Wq": Wq_p, "Wk": Wk_p, "Wv": Wv_p, "Wo": Wo_p,
            "cosO": np.ascontiguousarray(cosT[:, own_cols]),
            "sinO": np.ascontiguousarray(sinT[:, own_cols]),
            "cosK": np.ascontiguousarray(cosT[:, key_cols]),
            "sinK": np.ascontiguousarray(sinT[:, key_cols]),
            "masks": _make_masks(blocks),
        })

    nc = _get_nc()
    res = run_bass_kernel_spmd(nc, in_maps, core_ids=list(range(8)),
                               trace=trace)

    out = np.zeros((B, S, D), dtype=np.float32)
    for c, (b, blocks) in enumerate(core_blocks):
        o = res.results[c]["out"]
        for j, blk in enumerate(blocks):
            out[b, blk * QB:(blk + 1) * QB] = o[j * QB:(j + 1) * QB]
    return out, res


def kernel(**inputs):
    return run(inputs, trace=False)[0]
